# revision 1
# baseline (speedup 1.0000x reference)
"""Trainium2 Bass kernel for the 4-layer dense transformer (nn_DTransformer).

Self-contained: takes full unsharded inputs, shards across 8 NeuronCores
(sequence-parallel residual stream + head-sharded QK-merge + vocab-sharded
unembed), runs one SPMD Bass/Tile kernel, reassembles the full output.
"""
import sys

sys.path.insert(0, "/opt/trn_rl_repo")

import numpy as np
import ml_dtypes

import concourse.bass as bass
import concourse.mybir as mybir
import concourse.tile as tile
from concourse import bacc
from concourse.bass_utils import run_bass_kernel_spmd
from concourse.masks import make_identity

F32 = mybir.dt.float32
BF16 = mybir.dt.bfloat16
AF = mybir.ActivationFunctionType
ALU = mybir.AluOpType

L, D, H, DV, DM, VOC, NL = 2048, 768, 12, 64, 3072, 32000, 4
NC = 8
R = L // NC            # 256 rows per core
VC = VOC // NC         # 4000 vocab cols per core
ET = D // 128          # 6 feature tiles
JT = DM // 128         # 24 mlp tiles
MT = L // 128          # 16 m (key) tiles
LT = R // 128          # 2 local row tiles
NB = VC // 500         # 8 unembed col blocks
SCALE = float(1.0 / np.sqrt(np.float32(D)))

_CACHE = {}


def _build(analyze=False):
    nc = bacc.Bacc("TRN2", target_bir_lowering=False, debug=False, num_devices=NC)

    # ---------------- I/O ----------------
    e0 = nc.dram_tensor("e0", [R, D], F32, kind="ExternalInput")
    mq = nc.dram_tensor("mq", [NL, 3, D, D], BF16, kind="ExternalInput")
    mk = nc.dram_tensor("mk", [NL, 3, D, 384], BF16, kind="ExternalInput")
    lnp = nc.dram_tensor("lnp", [NL, D, 4], F32, kind="ExternalInput")
    lnf = nc.dram_tensor("lnf", [D, 2], F32, kind="ExternalInput")
    combo = nc.dram_tensor("combo", [NL, D, 87], BF16, kind="ExternalInput")
    woe = nc.dram_tensor("woe", [NL, 76, D], BF16, kind="ExternalInput")
    w1 = nc.dram_tensor("w1", [NL, D, DM], BF16, kind="ExternalInput")
    bm1c = nc.dram_tensor("bm1c", [NL, 128, JT], F32, kind="ExternalInput")
    w2 = nc.dram_tensor("w2", [NL, DM, D], BF16, kind="ExternalInput")
    bm2r = nc.dram_tensor("bm2r", [NL, 1, D], BF16, kind="ExternalInput")
    wue = nc.dram_tensor("wue", [D, VC], BF16, kind="ExternalInput")
    bur = nc.dram_tensor("bur", [1, VC], BF16, kind="ExternalInput")
    out = nc.dram_tensor("out", [L, VC], F32, kind="ExternalOutput")

    # ---------------- internal DRAM ----------------
    xnt_mine = [nc.dram_tensor(f"xnt_mine{i}", [D, R], BF16) for i in range(NL + 1)]
    xnt_all = [
        nc.dram_tensor(f"xnt_all{i}", [NC * D, R], BF16, addr_space="Shared")
        for i in range(NL + 1)
    ]
    mcontrib = [nc.dram_tensor(f"mcon{i}", [3 * D, 384], BF16) for i in range(NL)]
    m_all = [
        nc.dram_tensor(f"mall{i}", [NC * 3 * D, 384], BF16, addr_space="Shared")
        for i in range(NL)
    ]
    esc = nc.dram_tensor("esc", [L, VC], F32)
    denc = [nc.dram_tensor(f"denc{h}", [L // 4], F32) for h in range(4)]
    den_all = [
        nc.dram_tensor(f"den_all{h}", [L // 4], F32, addr_space="Shared")
        for h in range(4)
    ]

    RG = [list(range(NC))]

    with tile.TileContext(nc) as tc:
        with (
            tc.tile_pool(name="const", bufs=1) as cpool,
            tc.tile_pool(name="pers", bufs=1) as pers,
            tc.tile_pool(name="work", bufs=2) as work,
            tc.tile_pool(name="wt", bufs=1) as wtp,
            tc.tile_pool(name="mqp", bufs=1) as mqp,
            tc.tile_pool(name="mkp", bufs=2) as mkp,
            tc.tile_pool(name="mhp", bufs=3) as mhp,
            tc.tile_pool(name="etp", bufs=4) as etp,
            tc.tile_pool(name="w1p", bufs=6) as w1p,
            tc.tile_pool(name="w2p", bufs=24) as w2p,
            tc.tile_pool(name="gtp", bufs=24) as gtp,
            tc.tile_pool(name="wup", bufs=8) as wup,
            tc.tile_pool(name="eup", bufs=2) as eup,
            tc.tile_pool(name="scp", bufs=2) as scp,
            tc.tile_pool(name="ps", bufs=2, space="PSUM") as ps,
            tc.tile_pool(name="ps3", bufs=2, space="PSUM") as ps3,
        ):
            # constants
            ident = cpool.tile([128, 128], BF16)
            make_identity(nc, ident[:])
            identf = cpool.tile([128, 128], F32)
            make_identity(nc, identf[:])
            ones_row = cpool.tile([1, 128], BF16)   # K=1 matmul lhsT (all-ones)
            nc.vector.memset(ones_row[:], 1.0)

            # residual stream, f32: Y[:, lt*D + d], row l = lt*128 + p
            Y = pers.tile([128, LT * D], F32)
            for lt in range(LT):
                nc.sync.dma_start(Y[:, lt * D:(lt + 1) * D], e0[lt * 128:(lt + 1) * 128, :])

            xnTf = pers.tile([128, ET * L], BF16)   # gathered feature-major LN out

            def layernorm(pcol_g, pcol_b, want_rowmajor):
                """LN of Y chunk -> (lT bf16 [128, ET*R] feature-major, zn f32 or None).

                pcol_g/pcol_b: et -> sbuf [128, 1] f32 gamma/beta column accessors.
                """
                lT = pers.tile([128, ET * R], BF16, tag="lT")
                zn = pers.tile([128, LT * D], BF16, tag="zn", name="zn") if want_rowmajor else None
                for lt in range(LT):
                    ys = Y[:, lt * D:(lt + 1) * D]
                    mean = work.tile([128, 1], F32, tag="m1")
                    nc.vector.reduce_sum(mean[:], ys, axis=mybir.AxisListType.X)
                    nmean = work.tile([128, 1], F32, tag="m2")
                    nc.scalar.mul(nmean[:], mean[:], -1.0 / D)
                    cent = work.tile([128, D], BF16, tag="cent")
                    nc.vector.tensor_scalar_add(cent[:], ys, nmean[:])
                    sq = work.tile([128, D], BF16, tag="sq")
                    ssum = work.tile([128, 1], F32, tag="m3")
                    nc.scalar.activation(sq[:], cent[:], AF.Square, accum_out=ssum[:])
                    var = work.tile([128, 1], F32, tag="m4")
                    nc.scalar.mul(var[:], ssum[:], 1.0 / D)
                    std = work.tile([128, 1], F32, tag="m5")
                    nc.scalar.sqrt(std[:], var[:])
                    rstd = work.tile([128, 1], F32, tag="m6")
                    nc.vector.reciprocal(rstd[:], std[:])
                    norm = work.tile([128, D], BF16, tag="norm")
                    nc.vector.tensor_scalar_mul(norm[:], cent[:], rstd[:])
                    for et in range(ET):
                        pt = ps.tile([128, 128], BF16, tag="tr")
                        nc.tensor.transpose(pt[:], norm[:, et * 128:(et + 1) * 128], ident[:])
                        dst = lT[:, et * R + lt * 128: et * R + (lt + 1) * 128]
                        nc.vector.tensor_scalar(
                            dst, pt[:], pcol_g(et), pcol_b(et),
                            op0=ALU.mult, op1=ALU.add,
                        )
                if want_rowmajor:
                    for lt in range(LT):
                        for et in range(ET):
                            pt = ps.tile([128, 128], BF16, tag="tr")
                            nc.tensor.transpose(
                                pt[:], lT[:, et * R + lt * 128: et * R + (lt + 1) * 128],
                                ident[:],
                            )
                            nc.vector.tensor_copy(zn[:, lt * D + et * 128: lt * D + (et + 1) * 128], pt[:])
                return lT, zn

            def gather_lt(lT, mine_dram, all_dram):
                """DMA local feature-major chunk to DRAM, AllGather, load full."""
                for et in range(ET):
                    nc.sync.dma_start(
                        mine_dram[et * 128:(et + 1) * 128, :],
                        lT[:, et * R:(et + 1) * R],
                    )
                if analyze:
                    nc.sync.dma_start(all_dram[0:D, :], mine_dram[:])
                else:
                    nc.gpsimd.collective_compute(
                        "AllGather", ALU.bypass, replica_groups=RG,
                        ins=[mine_dram[:]], outs=[all_dram[:]],
                    )
                # all_dram rows: c*D + et*128 + p, cols l_local -> xnTf[p, et*L + c*R + l]
                v = all_dram[:, :].rearrange("(c e p) l -> e p c l", c=NC, e=ET, p=128)
                for et in range(ET):
                    dst = xnTf[:, et * L:(et + 1) * L].rearrange("p (c l) -> p c l", c=NC)
                    nc.sync.dma_start(dst, v[et])

            def compute_tT(i, h, hh, tT, lT):
                mh_sb = []
                for s in range(2):
                    g = 2 * h + s
                    kcore, p3 = g // 3, g % 3
                    mhs = mhp.tile([128, ET * 384], BF16, tag="mh", name="mhs")
                    base = kcore * 3 * D + p3 * D
                    nc.sync.dma_start(
                        mhs[:].rearrange("p (e c) -> p e c", c=384),
                        m_all[i][base:base + D, :].rearrange("(e p) c -> p e c", p=128),
                    )
                    mh_sb.append(mhs)
                for n6 in range(ET):
                    mhs = mh_sb[n6 // 3]
                    c0 = (n6 % 3) * 128
                    tp = ps3.tile([128, R], F32, tag="smm", name="tp")
                    for et in range(ET):
                        nc.tensor.matmul(
                            tp[:], mhs[:, et * 384 + c0: et * 384 + c0 + 128],
                            lT[:, et * R:(et + 1) * R],
                            start=(et == 0), stop=(et == ET - 1),
                        )
                    nc.vector.tensor_copy(
                        tT[:, n6 * 2 * R + hh * R: n6 * 2 * R + (hh + 1) * R], tp[:]
                    )

            def compute_m(i):
                for p3 in range(3):
                    mkt = mkp.tile([128, ET * 384], BF16, tag="mk", name="mkt")
                    nc.sync.dma_start(
                        mkt[:].rearrange("p (e c) -> p e c", c=384),
                        mk[i, p3].rearrange("(e p) c -> p e c", p=128),
                    )
                    qt = mqp.tile([128, ET * D], BF16, tag="mq", name="qt")
                    nc.sync.dma_start(
                        qt[:].rearrange("p (e c) -> p e c", c=D),
                        mq[i, p3].rearrange("(e p) c -> p e c", p=128),
                    )
                    for mt6 in range(ET):
                        mp = ps.tile([128, 384], F32, tag="mm", name="mp")
                        for kt in range(ET):
                            nc.tensor.matmul(
                                mp[:], qt[:, kt * D + mt6 * 128: kt * D + (mt6 + 1) * 128],
                                mkt[:, kt * 384:(kt + 1) * 384],
                                start=(kt == 0), stop=(kt == ET - 1),
                            )
                        mo = work.tile([128, 384], BF16, tag="mo", name="mo")
                        nc.vector.tensor_copy(mo[:], mp[:])
                        nc.sync.dma_start(
                            mcontrib[i][p3 * D + mt6 * 128: p3 * D + (mt6 + 1) * 128, :],
                            mo[:],
                        )
                if analyze:
                    nc.sync.dma_start(m_all[i][0:3 * D, :], mcontrib[i][:])
                else:
                    nc.gpsimd.collective_compute(
                        "AllGather", ALU.bypass, replica_groups=RG,
                        ins=[mcontrib[i][:]], outs=[m_all[i][:]],
                    )

            # layer 0's M first (its AG gates layer 0 attention), then LN1[0]
            # chunk + its AG, then prefetch M for layers 1..3 (their AGs drain
            # while layer 0 computes).
            compute_m(0)

            # ================= layers =================
            for i in range(NL):
                lnpt = wtp.tile([128, ET * 4], F32, tag="lnp")
                for et in range(ET):
                    nc.sync.dma_start(
                        lnpt[:, et * 4:(et + 1) * 4], lnp[i, et * 128:(et + 1) * 128, :]
                    )
                g1c = lambda et: lnpt[:, et * 4 + 0: et * 4 + 1]
                b1c = lambda et: lnpt[:, et * 4 + 1: et * 4 + 2]
                g2c = lambda et: lnpt[:, et * 4 + 2: et * 4 + 3]
                b2c = lambda et: lnpt[:, et * 4 + 3: et * 4 + 4]

                # ---- LN1 -> local feature-major + allgather ----
                lT, _ = layernorm(g1c, b1c, want_rowmajor=False)
                gather_lt(lT, xnt_mine[i], xnt_all[i])
                if i + 1 < NL:
                    compute_m(i + 1)

                # ---- combo: w (12) | v0 (11) | V11 (64) over all m ----
                cmb = wtp.tile([128, ET * 87], BF16, tag="cmb")
                for et in range(ET):
                    nc.sync.dma_start(
                        cmb[:, et * 87:(et + 1) * 87], combo[i, et * 128:(et + 1) * 128, :]
                    )
                w_sb = pers.tile([128, MT * 12], F32, tag="wsb")
                pvl = pers.tile([128, MT * 76], BF16, tag="pvl")
                for mt in range(MT):
                    cp = ps.tile([128, 87], F32, tag="pv")
                    for et in range(ET):
                        nc.tensor.matmul(
                            cp[:], xnTf[:, et * L + mt * 128: et * L + (mt + 1) * 128],
                            cmb[:, et * 87:(et + 1) * 87],
                            start=(et == 0), stop=(et == ET - 1),
                        )
                    nc.vector.tensor_copy(w_sb[:, mt * 12:(mt + 1) * 12], cp[:, 0:12])
                    nc.vector.tensor_copy(pvl[:, mt * 76: mt * 76 + 75], cp[:, 12:87])
                    nc.vector.memset(pvl[:, mt * 76 + 75: mt * 76 + 76], 1.0)

                # ---- attention heads ----
                ylm = pers.tile([128, LT * 76], BF16, tag="ylm")  # l-major y + ones col
                for lt in range(LT):
                    nc.vector.memset(ylm[:, lt * 76 + 75: lt * 76 + 76], 1.0)
                woet = wtp.tile([76, D], BF16, tag="woe")
                nc.sync.dma_start(woet[:], woe[i])

                for hpair in range(H // 2):
                    # pair heads (2hp, 2hp+1): shared lhsT in the S matmul ->
                    # one N=512 rhs, half the matmuls and LDWEIGHTS.
                    heads = (2 * hpair, 2 * hpair + 1)
                    tT = work.tile([128, ET * 2 * R], BF16, tag="tT", bufs=1, name="tT")
                    for hh, h in enumerate(heads):
                        compute_tT(i, h, hh, tT, lT)
                    # S^T per m-tile (both heads, N=512) -> exp -> PV accumulate
                    pvs = [ps.tile([128, R], F32, tag="pv", name="pv") for _ in range(2)]
                    prev_eTs = None
                    for mt in range(MT):
                        sp = ps3.tile([128, 2 * R], F32, tag="smm", name="sp")
                        for n6 in range(ET):
                            nc.tensor.matmul(
                                sp[:], xnTf[:, n6 * L + mt * 128: n6 * L + (mt + 1) * 128],
                                tT[:, n6 * 2 * R:(n6 + 1) * 2 * R],
                                start=(n6 == 0), stop=(n6 == ET - 1),
                            )
                        eTs = []
                        for hh, h in enumerate(heads):
                            eT = etp.tile([128, R], BF16, tag="eTm", name="eT")
                            nc.scalar.activation(
                                eT[:], sp[:, hh * R:(hh + 1) * R], AF.Exp,
                                bias=w_sb[:, mt * 12 + h: mt * 12 + h + 1], scale=SCALE,
                            )
                            eTs.append(eT)
                        # PV delayed one iteration so exp(mt-1) is done when PE gets here
                        if prev_eTs is not None:
                            for hh in range(2):
                                nc.tensor.matmul(
                                    pvs[hh][0:76, :],
                                    pvl[:, (mt - 1) * 76: mt * 76],
                                    prev_eTs[hh][:],
                                    start=(mt - 1 == 0), stop=False,
                                )
                        prev_eTs = eTs
                    for hh in range(2):
                        nc.tensor.matmul(
                            pvs[hh][0:76, :],
                            pvl[:, (MT - 1) * 76: MT * 76],
                            prev_eTs[hh][:],
                            start=False, stop=True,
                        )
                    for hh, h in enumerate(heads):
                        pv_sb = work.tile([76, R], F32, tag="pvsb", name="pv_sb")
                        nc.vector.tensor_copy(pv_sb[:], pvs[hh][0:76, :])
                        for lt in range(LT):
                            pvT = ps.tile([128, 76], F32, tag="tr", name="pvT")
                            nc.tensor.transpose(
                                pvT[:], pv_sb[:, lt * 128:(lt + 1) * 128], identf[0:76, 0:76]
                            )
                            recip = work.tile([128, 1], F32, tag="recip", name="recip")
                            nc.vector.reciprocal(recip[:], pvT[:, 75:76])
                            if h < H - 1:
                                nc.vector.tensor_scalar_mul(
                                    ylm[:, lt * 76 + h: lt * 76 + h + 1],
                                    pvT[:, h:h + 1], recip[:],
                                )
                            else:
                                nc.vector.tensor_scalar_mul(
                                    ylm[:, lt * 76 + 11: lt * 76 + 75],
                                    pvT[:, 11:75], recip[:],
                                )

                # ---- out-proj + residual: Y = 2Y + yT.T @ [Wo;bo] ----
                yT = pers.tile([76, LT * 128], BF16, tag="yT")
                for lt in range(LT):
                    ytp = ps.tile([128, 128], BF16, tag="tr")
                    nc.tensor.transpose(
                        ytp[0:76, :], ylm[:, lt * 76:(lt + 1) * 76], ident[:]
                    )
                    nc.vector.tensor_copy(yT[:, lt * 128:(lt + 1) * 128], ytp[0:76, :])
                for lt in range(LT):
                    for nb2 in range(2):
                        ap = ps.tile([128, 384], F32, tag="mm")
                        nc.tensor.matmul(
                            ap[:], yT[:, lt * 128:(lt + 1) * 128],
                            woet[:, nb2 * 384:(nb2 + 1) * 384],
                            start=True, stop=True,
                        )
                        ysl = Y[:, lt * D + nb2 * 384: lt * D + (nb2 + 1) * 384]
                        nc.vector.scalar_tensor_tensor(
                            ysl, ysl, 2.0, ap[:], op0=ALU.mult, op1=ALU.add
                        )

                # ---- MLP ----
                znT, zn = layernorm(g2c, b2c, want_rowmajor=True)
                w1t = [w1p.tile([128, DM], BF16, tag="w1", name="w1t") for _ in range(ET)]
                for et in range(ET):
                    nc.sync.dma_start(w1t[et][:], w1[i, et * 128:(et + 1) * 128, :])
                bm1t = wtp.tile([128, JT], F32, tag="bm1")
                nc.sync.dma_start(bm1t[:], bm1c[i])
                gts = []
                for jt in range(JT):
                    hp = ps.tile([128, R], F32, tag="mm")
                    for et in range(ET):
                        nc.tensor.matmul(
                            hp[:], w1t[et][:, jt * 128:(jt + 1) * 128],
                            znT[:, et * R:(et + 1) * R],
                            start=(et == 0), stop=(et == ET - 1),
                        )
                    gt = gtp.tile([128, R], BF16, tag="gT")
                    nc.scalar.activation(
                        gt[:], hp[:], AF.Gelu_apprx_tanh,
                        bias=bm1t[:, jt:jt + 1], scale=1.0,
                    )
                    gts.append(gt)
                w2t = [w2p.tile([128, D], BF16, tag="w2", name="w2t") for _ in range(JT)]
                for jt in range(JT):
                    nc.sync.dma_start(w2t[jt][:], w2[i, jt * 128:(jt + 1) * 128, :])
                bm2t = wtp.tile([1, D], BF16, tag="bm2")
                nc.sync.dma_start(bm2t[:], bm2r[i])
                for lt in range(LT):
                    nc.vector.tensor_add(
                        Y[:, lt * D:(lt + 1) * D], Y[:, lt * D:(lt + 1) * D],
                        zn[:, lt * D:(lt + 1) * D],
                    )
                    for nb2 in range(2):
                        mp2 = ps.tile([128, 384], F32, tag="mm")
                        for jt in range(JT):
                            nc.tensor.matmul(
                                mp2[:], gts[jt][:, lt * 128:(lt + 1) * 128],
                                w2t[jt][:, nb2 * 384:(nb2 + 1) * 384],
                                start=(jt == 0), stop=False,
                            )
                        nc.tensor.matmul(
                            mp2[:], ones_row[:, 0:128],
                            bm2t[:, nb2 * 384:(nb2 + 1) * 384],
                            start=False, stop=True,
                        )
                        ysl = Y[:, lt * D + nb2 * 384: lt * D + (nb2 + 1) * 384]
                        nc.vector.tensor_add(ysl, ysl, mp2[:])

            # ================= final LN + unembed + softmax =================
            lnft = wtp.tile([128, ET * 2], F32, tag="lnf")
            for et in range(ET):
                nc.sync.dma_start(lnft[:, et * 2:(et + 1) * 2], lnf[et * 128:(et + 1) * 128, :])
            gfc = lambda et: lnft[:, et * 2 + 0: et * 2 + 1]
            bfc = lambda et: lnft[:, et * 2 + 1: et * 2 + 2]
            lT, _ = layernorm(gfc, bfc, want_rowmajor=False)
            gather_lt(lT, xnt_mine[NL], xnt_all[NL])

            but = wtp.tile([1, VC], BF16, tag="bu")
            nc.sync.dma_start(but[:], bur[:])
            dens = pers.tile([128, MT * NB], F32, tag="dens")
            HM = MT // 4
            for hf in range(4):
                for nb in range(NB):
                    wut = [wup.tile([128, 500], BF16, tag="wu", name="wut") for _ in range(ET)]
                    for et in range(ET):
                        nc.sync.dma_start(
                            wut[et][:], wue[et * 128:(et + 1) * 128, nb * 500:(nb + 1) * 500]
                        )
                    for mt in range(hf * HM, (hf + 1) * HM):
                        up = ps.tile([128, 500], F32, tag="mm")
                        for et in range(ET):
                            nc.tensor.matmul(
                                up[:], xnTf[:, et * L + mt * 128: et * L + (mt + 1) * 128],
                                wut[et][:],
                                start=(et == 0), stop=False,
                            )
                        nc.tensor.matmul(
                            up[:], ones_row[:, 0:128], but[:, nb * 500:(nb + 1) * 500],
                            start=False, stop=True,
                        )
                        eu = eup.tile([128, 500], F32, tag="eu")
                        nc.scalar.activation(
                            eu[:], up[:], AF.Exp,
                            accum_out=dens[:, mt * NB + nb: mt * NB + nb + 1],
                        )
                        nc.sync.dma_start(
                            esc[mt * 128:(mt + 1) * 128, nb * 500:(nb + 1) * 500], eu[:]
                        )
                # reduce + allreduce + reciprocal + scale for this half
                dloc = pers.tile([128, HM], F32, tag="dloc", name="dloc")
                for j, mt in enumerate(range(hf * HM, (hf + 1) * HM)):
                    nc.vector.reduce_sum(
                        dloc[:, j:j + 1], dens[:, mt * NB:(mt + 1) * NB],
                        axis=mybir.AxisListType.X,
                    )
                    nc.sync.dma_start(denc[hf][j * 128:(j + 1) * 128], dloc[:, j:j + 1])
                if analyze:
                    nc.sync.dma_start(den_all[hf][:], denc[hf][:])
                else:
                    nc.gpsimd.collective_compute(
                        "AllReduce", ALU.add, replica_groups=RG,
                        ins=[denc[hf][:]], outs=[den_all[hf][:]],
                    )
                dall = pers.tile([128, HM], F32, tag="dall", name="dall")
                nc.sync.dma_start(dall[:], den_all[hf][:].rearrange("(m p) -> p m", p=128))
                drec = pers.tile([128, HM], F32, tag="drec", name="drec")
                nc.vector.reciprocal(drec[:], dall[:])
                for j, mt in enumerate(range(hf * HM, (hf + 1) * HM)):
                    for cb in range(8):
                        ei = scp.tile([128, 500], F32, tag="ei", name="ei")
                        nc.sync.dma_start(
                            ei[:], esc[mt * 128:(mt + 1) * 128, cb * 500:(cb + 1) * 500]
                        )
                        nc.vector.tensor_scalar_mul(ei[:], ei[:], drec[:, j:j + 1])
                        nc.sync.dma_start(
                            out[mt * 128:(mt + 1) * 128, cb * 500:(cb + 1) * 500], ei[:]
                        )

    nc.compile()
    return nc


def _prep_inputs(inputs):
    bf = ml_dtypes.bfloat16
    x = np.asarray(inputs["x"])
    E0 = (np.asarray(inputs["word_embed"])[x] + np.asarray(inputs["pos_embed"])).astype(np.float32)
    Wq, bq = np.asarray(inputs["Wq"]), np.asarray(inputs["bq"])
    Wk = np.asarray(inputs["Wk"])
    Wv, bv = np.asarray(inputs["Wv"]), np.asarray(inputs["bv"])
    Wo, bo = np.asarray(inputs["Wo"]), np.asarray(inputs["bo"])
    W1, bm1 = np.asarray(inputs["W1"]), np.asarray(inputs["bm1"])
    W2, bm2 = np.asarray(inputs["W2"]), np.asarray(inputs["bm2"])
    Wu, bu = np.asarray(inputs["Wu"]), np.asarray(inputs["bu"])

    lnp = np.stack(
        [np.asarray(inputs["g1"]), np.asarray(inputs["be1"]),
         np.asarray(inputs["g2"]), np.asarray(inputs["be2"])], axis=-1
    ).astype(np.float32)                                   # [NL, D, 4]
    lnf = np.stack([np.asarray(inputs["gf"]), np.asarray(inputs["bef"])], -1).astype(np.float32)

    combo = np.zeros((NL, D, 87), np.float32)
    for i in range(NL):
        for h in range(H):
            combo[i, :, h] = (Wk[i, h] @ bq[i, h]) * SCALE   # u_scaled
        combo[i, :, 12:23] = Wv[i, :11, :, 0].transpose(1, 0)
        combo[i, :, 23:87] = Wv[i, 11]
    woe = np.zeros((NL, 76, D), np.float32)
    for i in range(NL):
        bv_flat = np.concatenate([bv[i, :11, 0], bv[i, 11]])
        woe[i, :75] = Wo[i, :75]
        woe[i, 75] = bo[i] + bv_flat @ Wo[i, :75]
    bm1c = bm1.reshape(NL, JT, 128).transpose(0, 2, 1).astype(np.float32)

    in_maps = []
    for k in range(NC):
        mqk = np.zeros((NL, 3, D, D), np.float32)
        mkk = np.zeros((NL, 3, D, 384), np.float32)
        for p3 in range(3):
            g = 3 * k + p3
            h, s = g // 2, g % 2
            for i in range(NL):
                mqk[i, p3] = Wq[i, h].T
                mkk[i, p3] = Wk[i, h][s * 384:(s + 1) * 384, :].T
        in_maps.append({
            "e0": E0[k * R:(k + 1) * R],
            "mq": mqk.astype(bf),
            "mk": mkk.astype(bf),
            "lnp": lnp,
            "lnf": lnf,
            "combo": combo.astype(bf),
            "woe": woe.astype(bf),
            "w1": W1.astype(bf),
            "bm1c": bm1c,
            "w2": W2.astype(bf),
            "bm2r": bm2.reshape(NL, 1, D).astype(bf),
            "wue": np.ascontiguousarray(Wu[:, k * VC:(k + 1) * VC]).astype(bf),
            "bur": np.ascontiguousarray(bu[None, k * VC:(k + 1) * VC]).astype(bf),
        })
    return in_maps


def _run(inputs, **kw):
    if "nc" not in _CACHE:
        _CACHE["nc"] = _build()
    nc = _CACHE["nc"]
    in_maps = _prep_inputs(inputs)
    res = run_bass_kernel_spmd(nc, in_maps, list(range(NC)), **kw)
    outp = np.concatenate([res.results[k]["out"] for k in range(NC)], axis=1)
    return outp.astype(np.float32), res


def kernel(**inputs):
    outp, _ = _run(inputs)
    return outp



# revision 11
# speedup vs baseline: 2.0955x; 2.0955x over previous
"""Trainium2 Bass kernel for the 4-layer dense transformer (nn_DTransformer).

Self-contained: takes full unsharded inputs, shards across 8 NeuronCores
(sequence-parallel residual stream + head-sharded QK-merge + vocab-sharded
unembed), runs one SPMD Bass/Tile kernel, reassembles the full output.

v2: fp8(e4m3) tensor cores for the dominant matmuls (scores via DoubleRow,
QK-merge, MLP up-proj, unembed), SBUF-resident unembed weights with per-row-
tile softmax denominators (no DRAM spill of the exp matrix), halved collective
payloads. Activation scales are folded host-side (see _prep_inputs) and undone
in the activation-function `scale` arguments.
"""
import sys

sys.path.insert(0, "/opt/trn_rl_repo")

import numpy as np
import ml_dtypes

import concourse.bass as bass
import concourse.mybir as mybir
import concourse.tile as tile
from concourse import bacc
from concourse.bass_utils import run_bass_kernel_spmd
from concourse.masks import make_identity

F32 = mybir.dt.float32
BF16 = mybir.dt.bfloat16
F8 = mybir.dt.float8e4
AF = mybir.ActivationFunctionType
ALU = mybir.AluOpType
DR = mybir.MatmulPerfMode.DoubleRow

L, D, H, DV, DM, VOC, NL = 2048, 768, 12, 64, 3072, 32000, 4
NC = 8
R = L // NC            # 256 rows per core
VC = VOC // NC         # 4000 vocab cols per core
ET = D // 128          # 6 feature tiles
JT = DM // 128         # 24 mlp tiles
MT = L // 128          # 16 m (key) tiles
LT = R // 128          # 2 local row tiles
NB = VC // 500         # 8 unembed col blocks
SCALE = float(1.0 / np.sqrt(np.float32(D)))

# fp8 pre-scales (host-side; undone in activation `scale` args)
CL = 4.0               # LN1 / final-LN gamma,beta (lT for scores+unembed)
CW = 8.0               # LN2 gamma,beta (znT for MLP)
CM = 8.0               # each of mq, mk  -> M carries CM*CM = 64
CC = 64.0              # combo v-columns
CCW = 1024.0           # combo w-columns
CU = 64.0              # unembed / MLP-up weight scale

_CACHE = {}


def _pair(t, w, c, lo, hi):
    """[128, 6*w] et-major tile -> [128, 2, hi-lo] fp8 pair view for
    et = (2c, 2c+1), used as a DoubleRow matmul operand."""
    return t[:, 2 * c * w:(2 * c + 2) * w].rearrange("p (e n) -> p e n", e=2)[:, :, lo:hi]


def _build(analyze=False, reps=1):
    nc = bacc.Bacc("TRN2", target_bir_lowering=False, debug=False, num_devices=NC)

    # ---------------- I/O ----------------
    e0 = nc.dram_tensor("e0", [R, D], F32, kind="ExternalInput")
    mq = nc.dram_tensor("mq", [NL, 3, D, D], F8, kind="ExternalInput")
    mk = nc.dram_tensor("mk", [NL, 3, D, 384], F8, kind="ExternalInput")
    lnp = nc.dram_tensor("lnp", [NL, D, 6], F32, kind="ExternalInput")
    lnf = nc.dram_tensor("lnf", [D, 2], F32, kind="ExternalInput")
    combo = nc.dram_tensor("combo", [NL, D, 87], F8, kind="ExternalInput")
    woe = nc.dram_tensor("woe", [NL, 76, D], BF16, kind="ExternalInput")
    w1 = nc.dram_tensor("w1", [NL, D, DM], F8, kind="ExternalInput")
    bm1c = nc.dram_tensor("bm1c", [NL, 128, JT], F32, kind="ExternalInput")
    w2 = nc.dram_tensor("w2", [NL, DM, D], BF16, kind="ExternalInput")
    bm2r = nc.dram_tensor("bm2r", [NL, 1, D], BF16, kind="ExternalInput")
    wue = nc.dram_tensor("wue", [D, VC], F8, kind="ExternalInput")
    bur = nc.dram_tensor("bur", [1, VC], BF16, kind="ExternalInput")
    out = nc.dram_tensor("out", [L, VC], F32, kind="ExternalOutput")

    # ---------------- internal DRAM ----------------
    xnt_mine = [nc.dram_tensor(f"xnt_mine{i}", [D, R], F8) for i in range(NL + 1)]
    xnt_all = [
        nc.dram_tensor(f"xnt_all{i}", [NC * D, R], F8, addr_space="Shared")
        for i in range(NL + 1)
    ]
    mcontrib = [nc.dram_tensor(f"mcon{i}", [3 * D, 384], F8) for i in range(NL)]
    m_all = [
        nc.dram_tensor(f"mall{i}", [NC * 3 * D, 384], F8, addr_space="Shared")
        for i in range(NL)
    ]
    denm = [nc.dram_tensor(f"denm{g}", [512], F32) for g in range(4)]
    den_allm = [
        nc.dram_tensor(f"den_allm{g}", [512], F32, addr_space="Shared")
        for g in range(4)
    ]

    RG = [list(range(NC))]

    with tile.TileContext(nc) as tc:
      def _one_rep():
            with (
                tc.tile_pool(name="const", bufs=1) as cpool,
                tc.tile_pool(name="pers", bufs=1) as pers,
                tc.tile_pool(name="work", bufs=2) as work,
                tc.tile_pool(name="wt", bufs=1) as wtp,
                tc.tile_pool(name="mqp", bufs=1) as mqp,
                tc.tile_pool(name="mkp", bufs=2) as mkp,
                tc.tile_pool(name="mhp", bufs=3) as mhp,
                tc.tile_pool(name="etp", bufs=4) as etp,
                tc.tile_pool(name="w1p", bufs=6) as w1p,
                tc.tile_pool(name="w2p", bufs=24) as w2p,
                tc.tile_pool(name="gtp", bufs=24) as gtp,
                tc.tile_pool(name="eup", bufs=36) as eup,
                tc.tile_pool(name="scp", bufs=3) as scp,
                tc.tile_pool(name="ps", bufs=2, space="PSUM") as ps,
                tc.tile_pool(name="ps3", bufs=2, space="PSUM") as ps3,
            ):
                # constants
                ident = cpool.tile([128, 128], BF16)
                make_identity(nc, ident[:])
                identf = cpool.tile([128, 128], F32)
                make_identity(nc, identf[:])
                ones_row = cpool.tile([1, 128], BF16)   # K=1 matmul lhsT (all-ones)
                nc.vector.memset(ones_row[:], 1.0)

                # residual stream, f32: Y[:, lt*D + d], row l = lt*128 + p
                Y = pers.tile([128, LT * D], F32)
                for lt in range(LT):
                    nc.sync.dma_start(Y[:, lt * D:(lt + 1) * D], e0[lt * 128:(lt + 1) * 128, :])

                xnTf = pers.tile([128, ET * L], F8)   # gathered feature-major LN out

                def layernorm(pcol_g, pcol_b, raw_cols, want_rowmajor):
                    """LN of Y chunk -> (lT fp8 [128, ET*R] feature-major, zn bf16 or None).

                    pcol_g/pcol_b: et -> sbuf [128, 1] f32 scaled gamma/beta columns.
                    raw_cols: (g_raw, b_raw) accessors for the unscaled row-major copy.
                    """
                    lT = pers.tile([128, ET * R], F8, tag="lT", name="lT")
                    lTb = pers.tile([128, ET * R], BF16, tag="lTb", name="lTb") if want_rowmajor else None
                    zn = pers.tile([128, LT * D], BF16, tag="zn", name="zn") if want_rowmajor else None
                    for lt in range(LT):
                        ys = Y[:, lt * D:(lt + 1) * D]
                        mean = work.tile([128, 1], F32, tag="m1")
                        nc.vector.reduce_sum(mean[:], ys, axis=mybir.AxisListType.X)
                        nmean = work.tile([128, 1], F32, tag="m2")
                        nc.scalar.mul(nmean[:], mean[:], -1.0 / D)
                        cent = work.tile([128, D], BF16, tag="cent")
                        nc.vector.tensor_scalar_add(cent[:], ys, nmean[:])
                        sq = work.tile([128, D], BF16, tag="sq")
                        ssum = work.tile([128, 1], F32, tag="m3")
                        nc.scalar.activation(sq[:], cent[:], AF.Square, accum_out=ssum[:])
                        var = work.tile([128, 1], F32, tag="m4")
                        nc.scalar.mul(var[:], ssum[:], 1.0 / D)
                        std = work.tile([128, 1], F32, tag="m5")
                        nc.scalar.sqrt(std[:], var[:])
                        rstd = work.tile([128, 1], F32, tag="m6")
                        nc.vector.reciprocal(rstd[:], std[:])
                        norm = work.tile([128, D], BF16, tag="norm")
                        nc.vector.tensor_scalar_mul(norm[:], cent[:], rstd[:])
                        for et in range(ET):
                            pt = ps.tile([128, 128], BF16, tag="tr")
                            nc.tensor.transpose(pt[:], norm[:, et * 128:(et + 1) * 128], ident[:])
                            dst = lT[:, et * R + lt * 128: et * R + (lt + 1) * 128]
                            nc.vector.tensor_scalar(
                                dst, pt[:], pcol_g(et), pcol_b(et),
                                op0=ALU.mult, op1=ALU.add,
                            )
                            if want_rowmajor:
                                g_raw, b_raw = raw_cols
                                dstb = lTb[:, et * R + lt * 128: et * R + (lt + 1) * 128]
                                nc.vector.tensor_scalar(
                                    dstb, pt[:], g_raw(et), b_raw(et),
                                    op0=ALU.mult, op1=ALU.add,
                                )
                    if want_rowmajor:
                        for lt in range(LT):
                            for et in range(ET):
                                pt = ps.tile([128, 128], BF16, tag="tr")
                                nc.tensor.transpose(
                                    pt[:], lTb[:, et * R + lt * 128: et * R + (lt + 1) * 128],
                                    ident[:],
                                )
                                nc.vector.tensor_copy(zn[:, lt * D + et * 128: lt * D + (et + 1) * 128], pt[:])
                    return lT, zn

                def gather_lt(lT, mine_dram, all_dram):
                    """DMA local feature-major chunk to DRAM, AllGather, load full."""
                    for et in range(ET):
                        nc.sync.dma_start(
                            mine_dram[et * 128:(et + 1) * 128, :],
                            lT[:, et * R:(et + 1) * R],
                        )
                    if analyze:
                        nc.sync.dma_start(all_dram[0:D, :], mine_dram[:])
                    else:
                        nc.gpsimd.collective_compute(
                            "AllGather", ALU.bypass, replica_groups=RG,
                            ins=[mine_dram[:]], outs=[all_dram[:]],
                        )
                    # all_dram rows: c*D + et*128 + p, cols l_local -> xnTf[p, et*L + c*R + l]
                    v = all_dram[:, :].rearrange("(c e p) l -> e p c l", c=NC, e=ET, p=128)
                    for et in range(ET):
                        dst = xnTf[:, et * L:(et + 1) * L].rearrange("p (c l) -> p c l", c=NC)
                        nc.sync.dma_start(dst, v[et])

                def compute_tT(i, h, hh, tT, lT):
                    mh_sb = []
                    for s in range(2):
                        g = 2 * h + s
                        kcore, p3 = g // 3, g % 3
                        mhs = mhp.tile([128, ET * 384], F8, tag="mh", name="mhs")
                        base = kcore * 3 * D + p3 * D
                        nc.sync.dma_start(
                            mhs[:].rearrange("p (e c) -> p e c", c=384),
                            m_all[i][base:base + D, :].rearrange("(e p) c -> p e c", p=128),
                        )
                        mh_sb.append(mhs)
                    for n6 in range(ET):
                        mhs = mh_sb[n6 // 3]
                        c0 = (n6 % 3) * 128
                        tp = ps3.tile([128, R], F32, tag="smm", name="tp")
                        for et in range(ET):
                            nc.tensor.matmul(
                                tp[:], mhs[:, et * 384 + c0: et * 384 + c0 + 128],
                                lT[:, et * R:(et + 1) * R],
                                start=(et == 0), stop=(et == ET - 1),
                            )
                        nc.vector.tensor_copy(
                            tT[:, n6 * 2 * R + hh * R: n6 * 2 * R + (hh + 1) * R], tp[:]
                        )

                def compute_m(i):
                    for p3 in range(3):
                        mkt = mkp.tile([128, ET * 384], F8, tag="mk", name="mkt")
                        nc.sync.dma_start(
                            mkt[:].rearrange("p (e c) -> p e c", c=384),
                            mk[i, p3].rearrange("(e p) c -> p e c", p=128),
                        )
                        qt = mqp.tile([128, ET * D], F8, tag="mq", name="qt")
                        nc.sync.dma_start(
                            qt[:].rearrange("p (e c) -> p e c", c=D),
                            mq[i, p3].rearrange("(e p) c -> p e c", p=128),
                        )
                        for mt6 in range(ET):
                            mp = ps.tile([128, 384], F32, tag="mm", name="mp")
                            for kc in range(3):
                                nc.tensor.matmul(
                                    mp[:], _pair(qt, D, kc, mt6 * 128, (mt6 + 1) * 128),
                                    _pair(mkt, 384, kc, 0, 384),
                                    start=(kc == 0), stop=(kc == 2), perf_mode=DR,
                                )
                            mo = work.tile([128, 384], F8, tag="mo", name="mo")
                            nc.vector.tensor_copy(mo[:], mp[:])
                            nc.sync.dma_start(
                                mcontrib[i][p3 * D + mt6 * 128: p3 * D + (mt6 + 1) * 128, :],
                                mo[:],
                            )
                    if analyze:
                        nc.sync.dma_start(m_all[i][0:3 * D, :], mcontrib[i][:])
                    else:
                        nc.gpsimd.collective_compute(
                            "AllGather", ALU.bypass, replica_groups=RG,
                            ins=[mcontrib[i][:]], outs=[m_all[i][:]],
                        )

                # layer 0's M first (its AG gates layer 0 attention), then LN1[0]
                # chunk + its AG, then prefetch M for layers 1..3 (their AGs drain
                # while layer 0 computes).
                compute_m(0)

                # ================= layers =================
                for i in range(NL):
                    lnpt = wtp.tile([128, ET * 6], F32, tag="lnp")
                    for et in range(ET):
                        nc.sync.dma_start(
                            lnpt[:, et * 6:(et + 1) * 6], lnp[i, et * 128:(et + 1) * 128, :]
                        )
                    g1c = lambda et: lnpt[:, et * 6 + 0: et * 6 + 1]
                    b1c = lambda et: lnpt[:, et * 6 + 1: et * 6 + 2]
                    g2c = lambda et: lnpt[:, et * 6 + 2: et * 6 + 3]
                    b2c = lambda et: lnpt[:, et * 6 + 3: et * 6 + 4]
                    g2r = lambda et: lnpt[:, et * 6 + 4: et * 6 + 5]
                    b2r = lambda et: lnpt[:, et * 6 + 5: et * 6 + 6]

                    # ---- LN1 -> local feature-major + allgather ----
                    lT, _ = layernorm(g1c, b1c, None, want_rowmajor=False)
                    gather_lt(lT, xnt_mine[i], xnt_all[i])
                    if i + 1 < NL:
                        compute_m(i + 1)

                    # ---- combo: w (12) | v0 (11) | V11 (64) over all m ----
                    cmb = wtp.tile([128, ET * 87], F8, tag="cmb")
                    for et in range(ET):
                        nc.sync.dma_start(
                            cmb[:, et * 87:(et + 1) * 87], combo[i, et * 128:(et + 1) * 128, :]
                        )
                    w_sb = pers.tile([128, MT * 12], F32, tag="wsb")
                    pvl = pers.tile([128, MT * 76], BF16, tag="pvl")
                    for mt in range(MT):
                        cp = ps.tile([128, 87], F32, tag="pv")
                        for et in range(ET):
                            nc.tensor.matmul(
                                cp[:], xnTf[:, et * L + mt * 128: et * L + (mt + 1) * 128],
                                cmb[:, et * 87:(et + 1) * 87],
                                start=(et == 0), stop=(et == ET - 1),
                            )
                        nc.scalar.mul(w_sb[:, mt * 12:(mt + 1) * 12], cp[:, 0:12], 1.0 / (CCW * CL))
                        nc.scalar.mul(pvl[:, mt * 76: mt * 76 + 75], cp[:, 12:87], 1.0 / (CC * CL))
                        nc.vector.memset(pvl[:, mt * 76 + 75: mt * 76 + 76], 1.0)

                    # ---- attention heads ----
                    ylm = pers.tile([128, LT * 76], BF16, tag="ylm")  # l-major y + ones col
                    for lt in range(LT):
                        nc.vector.memset(ylm[:, lt * 76 + 75: lt * 76 + 76], 1.0)
                    woet = wtp.tile([76, D], BF16, tag="woe")
                    nc.sync.dma_start(woet[:], woe[i])

                    for hpair in range(H // 2):
                        # pair heads (2hp, 2hp+1): shared lhsT in the S matmul ->
                        # one N=512 rhs, fp8 DoubleRow halves the instruction count.
                        heads = (2 * hpair, 2 * hpair + 1)
                        tT = work.tile([128, ET * 2 * R], F8, tag="tT", bufs=1, name="tT")
                        for hh, h in enumerate(heads):
                            compute_tT(i, h, hh, tT, lT)
                        pvs = [ps.tile([128, R], F32, tag="pv", name="pv") for _ in range(2)]
                        prev_eTs = None
                        for mt in range(MT):
                            sp = ps3.tile([128, 2 * R], F32, tag="smm", name="sp")
                            for kc in range(3):
                                nc.tensor.matmul(
                                    sp[:], _pair(xnTf, L, kc, mt * 128, (mt + 1) * 128),
                                    _pair(tT, 2 * R, kc, 0, 2 * R),
                                    start=(kc == 0), stop=(kc == 2), perf_mode=DR,
                                )
                            eTs = []
                            for hh, h in enumerate(heads):
                                eT = etp.tile([128, R], BF16, tag="eTm", name="eT")
                                nc.scalar.activation(
                                    eT[:], sp[:, hh * R:(hh + 1) * R], AF.Exp,
                                    bias=w_sb[:, mt * 12 + h: mt * 12 + h + 1],
                                    scale=SCALE / (CL * CL * CM * CM),
                                )
                                eTs.append(eT)
                            # PV delayed one iteration so exp(mt-1) is done when PE gets here
                            if prev_eTs is not None:
                                for hh in range(2):
                                    nc.tensor.matmul(
                                        pvs[hh][0:76, :],
                                        pvl[:, (mt - 1) * 76: mt * 76],
                                        prev_eTs[hh][:],
                                        start=(mt - 1 == 0), stop=False,
                                    )
                            prev_eTs = eTs
                        for hh in range(2):
                            nc.tensor.matmul(
                                pvs[hh][0:76, :],
                                pvl[:, (MT - 1) * 76: MT * 76],
                                prev_eTs[hh][:],
                                start=False, stop=True,
                            )
                        for hh, h in enumerate(heads):
                            pv_sb = work.tile([76, R], F32, tag="pvsb", name="pv_sb")
                            nc.vector.tensor_copy(pv_sb[:], pvs[hh][0:76, :])
                            for lt in range(LT):
                                pvT = ps.tile([128, 76], F32, tag="tr", name="pvT")
                                nc.tensor.transpose(
                                    pvT[:], pv_sb[:, lt * 128:(lt + 1) * 128], identf[0:76, 0:76]
                                )
                                recip = work.tile([128, 1], F32, tag="recip", name="recip")
                                nc.vector.reciprocal(recip[:], pvT[:, 75:76])
                                if h < H - 1:
                                    nc.vector.tensor_scalar_mul(
                                        ylm[:, lt * 76 + h: lt * 76 + h + 1],
                                        pvT[:, h:h + 1], recip[:],
                                    )
                                else:
                                    nc.vector.tensor_scalar_mul(
                                        ylm[:, lt * 76 + 11: lt * 76 + 75],
                                        pvT[:, 11:75], recip[:],
                                    )

                    # ---- out-proj + residual: Y = 2Y + yT.T @ [Wo;bo] ----
                    yT = pers.tile([76, LT * 128], BF16, tag="yT")
                    for lt in range(LT):
                        ytp = ps.tile([128, 128], BF16, tag="tr")
                        nc.tensor.transpose(
                            ytp[0:76, :], ylm[:, lt * 76:(lt + 1) * 76], ident[:]
                        )
                        nc.vector.tensor_copy(yT[:, lt * 128:(lt + 1) * 128], ytp[0:76, :])
                    for lt in range(LT):
                        for nb2 in range(2):
                            ap = ps.tile([128, 384], F32, tag="mm")
                            nc.tensor.matmul(
                                ap[:], yT[:, lt * 128:(lt + 1) * 128],
                                woet[:, nb2 * 384:(nb2 + 1) * 384],
                                start=True, stop=True,
                            )
                            ysl = Y[:, lt * D + nb2 * 384: lt * D + (nb2 + 1) * 384]
                            nc.vector.scalar_tensor_tensor(
                                ysl, ysl, 2.0, ap[:], op0=ALU.mult, op1=ALU.add
                            )

                    # ---- MLP ----
                    znT, zn = layernorm(g2c, b2c, (g2r, b2r), want_rowmajor=True)
                    w1t = [w1p.tile([128, DM], F8, tag="w1", name="w1t") for _ in range(ET)]
                    for et in range(ET):
                        nc.sync.dma_start(w1t[et][:], w1[i, et * 128:(et + 1) * 128, :])
                    bm1t = wtp.tile([128, JT], F32, tag="bm1")
                    nc.sync.dma_start(bm1t[:], bm1c[i])
                    gts = []
                    for jt in range(JT):
                        hp = ps.tile([128, R], F32, tag="mm")
                        for et in range(ET):
                            nc.tensor.matmul(
                                hp[:], w1t[et][:, jt * 128:(jt + 1) * 128],
                                znT[:, et * R:(et + 1) * R],
                                start=(et == 0), stop=(et == ET - 1),
                            )
                        gt = gtp.tile([128, R], BF16, tag="gT")
                        nc.scalar.activation(
                            gt[:], hp[:], AF.Gelu_apprx_tanh,
                            bias=bm1t[:, jt:jt + 1], scale=1.0 / (CW * CU),
                        )
                        gts.append(gt)
                    w2t = [w2p.tile([128, D], BF16, tag="w2", name="w2t") for _ in range(JT)]
                    for jt in range(JT):
                        nc.sync.dma_start(w2t[jt][:], w2[i, jt * 128:(jt + 1) * 128, :])
                    bm2t = wtp.tile([1, D], BF16, tag="bm2")
                    nc.sync.dma_start(bm2t[:], bm2r[i])
                    for lt in range(LT):
                        nc.vector.tensor_add(
                            Y[:, lt * D:(lt + 1) * D], Y[:, lt * D:(lt + 1) * D],
                            zn[:, lt * D:(lt + 1) * D],
                        )
                        for nb2 in range(2):
                            mp2 = ps.tile([128, 384], F32, tag="mm")
                            for jt in range(JT):
                                nc.tensor.matmul(
                                    mp2[:], gts[jt][:, lt * 128:(lt + 1) * 128],
                                    w2t[jt][:, nb2 * 384:(nb2 + 1) * 384],
                                    start=(jt == 0), stop=False,
                                )
                            nc.tensor.matmul(
                                mp2[:], ones_row[:, 0:128],
                                bm2t[:, nb2 * 384:(nb2 + 1) * 384],
                                start=False, stop=True,
                            )
                            ysl = Y[:, lt * D + nb2 * 384: lt * D + (nb2 + 1) * 384]
                            nc.vector.tensor_add(ysl, ysl, mp2[:])

                # ============ final LN + unembed + fused softmax ============
                lnft = wtp.tile([128, ET * 2], F32, tag="lnf")
                for et in range(ET):
                    nc.sync.dma_start(lnft[:, et * 2:(et + 1) * 2], lnf[et * 128:(et + 1) * 128, :])
                gfc = lambda et: lnft[:, et * 2 + 0: et * 2 + 1]
                bfc = lambda et: lnft[:, et * 2 + 1: et * 2 + 2]
                lT, _ = layernorm(gfc, bfc, None, want_rowmajor=False)
                gather_lt(lT, xnt_mine[NL], xnt_all[NL])

                # unembed weights resident in SBUF (fp8), single pass over Wu
                wu_sb = pers.tile([128, ET * VC], F8, tag="wusb")
                for et in range(ET):
                    nc.sync.dma_start(wu_sb[:, et * VC:(et + 1) * VC], wue[et * 128:(et + 1) * 128, :])
                but = wtp.tile([1, VC], BF16, tag="bu")
                nc.sync.dma_start(but[:], bur[:])

                for grp in range(4):
                    grp_eus = {}
                    for j in range(4):
                        mt = grp * 4 + j
                        dpart = work.tile([128, NB], F32, tag="dpart", name="dpart")
                        eus = []
                        for nb in range(NB):
                            up = ps3.tile([128, 500], F32, tag="smm", name="up")
                            for kc in range(3):
                                nc.tensor.matmul(
                                    up[:], _pair(xnTf, L, kc, mt * 128, (mt + 1) * 128),
                                    _pair(wu_sb, VC, kc, nb * 500, (nb + 1) * 500),
                                    start=(kc == 0), stop=False, perf_mode=DR,
                                )
                            nc.tensor.matmul(
                                up[:], ones_row[:, 0:128], but[:, nb * 500:(nb + 1) * 500],
                                start=False, stop=True,
                            )
                            eu = eup.tile([128, 500], BF16, tag="eu", name="eu")
                            nc.scalar.activation(
                                eu[:], up[:], AF.Exp, scale=1.0 / (CL * CU),
                                accum_out=dpart[:, nb:nb + 1],
                            )
                            eus.append(eu)
                        grp_eus[j] = eus
                        dloc = work.tile([128, 1], F32, tag="dloc", name="dloc")
                        nc.vector.reduce_sum(dloc[:], dpart[:], axis=mybir.AxisListType.X)
                        nc.sync.dma_start(denm[grp][j * 128:(j + 1) * 128], dloc[:])
                    if analyze:
                        nc.sync.dma_start(den_allm[grp][:], denm[grp][:])
                    else:
                        nc.gpsimd.collective_compute(
                            "AllReduce", ALU.add, replica_groups=RG,
                            ins=[denm[grp][:]], outs=[den_allm[grp][:]],
                        )
                    dall = work.tile([128, 4], F32, tag="dall", name="dall")
                    nc.sync.dma_start(dall[:], den_allm[grp][:].rearrange("(m p) -> p m", p=128))
                    drec = work.tile([128, 4], F32, tag="drec", name="drec")
                    nc.vector.reciprocal(drec[:], dall[:])
                    for j in range(4):
                        mt = grp * 4 + j
                        for nb in range(NB):
                            sout = scp.tile([128, 500], F32, tag="so", name="sout")
                            nc.vector.tensor_scalar_mul(
                                sout[:], grp_eus[j][nb][:], drec[:, j:j + 1]
                            )
                            nc.sync.dma_start(
                                out[mt * 128:(mt + 1) * 128, nb * 500:(nb + 1) * 500], sout[:]
                            )

      for _rep in range(reps):
          _one_rep()

    nc.compile()
    return nc


def _prep_inputs(inputs):
    bf = ml_dtypes.bfloat16
    f8 = mybir.dt.np(F8)
    x = np.asarray(inputs["x"])
    E0 = (np.asarray(inputs["word_embed"])[x] + np.asarray(inputs["pos_embed"])).astype(np.float32)
    Wq, bq = np.asarray(inputs["Wq"]), np.asarray(inputs["bq"])
    Wk = np.asarray(inputs["Wk"])
    Wv, bv = np.asarray(inputs["Wv"]), np.asarray(inputs["bv"])
    Wo, bo = np.asarray(inputs["Wo"]), np.asarray(inputs["bo"])
    W1, bm1 = np.asarray(inputs["W1"]), np.asarray(inputs["bm1"])
    W2, bm2 = np.asarray(inputs["W2"]), np.asarray(inputs["bm2"])
    Wu, bu = np.asarray(inputs["Wu"]), np.asarray(inputs["bu"])

    g1 = np.asarray(inputs["g1"]); be1 = np.asarray(inputs["be1"])
    g2 = np.asarray(inputs["g2"]); be2 = np.asarray(inputs["be2"])
    lnp = np.stack(
        [CL * g1, CL * be1, CW * g2, CW * be2, g2, be2], axis=-1
    ).astype(np.float32)                                   # [NL, D, 6]
    lnf = np.stack(
        [CL * np.asarray(inputs["gf"]), CL * np.asarray(inputs["bef"])], -1
    ).astype(np.float32)

    combo = np.zeros((NL, D, 87), np.float32)
    for i in range(NL):
        for h in range(H):
            combo[i, :, h] = (Wk[i, h] @ bq[i, h]) * SCALE * CCW  # u_scaled
        combo[i, :, 12:23] = Wv[i, :11, :, 0].transpose(1, 0) * CC
        combo[i, :, 23:87] = Wv[i, 11] * CC
    woe = np.zeros((NL, 76, D), np.float32)
    for i in range(NL):
        bv_flat = np.concatenate([bv[i, :11, 0], bv[i, 11]])
        woe[i, :75] = Wo[i, :75]
        woe[i, 75] = bo[i] + bv_flat @ Wo[i, :75]
    bm1c = bm1.reshape(NL, JT, 128).transpose(0, 2, 1).astype(np.float32)

    in_maps = []
    for k in range(NC):
        mqk = np.zeros((NL, 3, D, D), np.float32)
        mkk = np.zeros((NL, 3, D, 384), np.float32)
        for p3 in range(3):
            g = 3 * k + p3
            h, s = g // 2, g % 2
            for i in range(NL):
                mqk[i, p3] = CM * Wq[i, h].T
                mkk[i, p3] = CM * Wk[i, h][s * 384:(s + 1) * 384, :].T
        in_maps.append({
            "e0": E0[k * R:(k + 1) * R],
            "mq": mqk.astype(f8),
            "mk": mkk.astype(f8),
            "lnp": lnp,
            "lnf": lnf,
            "combo": combo.astype(f8),
            "woe": woe.astype(bf),
            "w1": (CU * W1).astype(f8),
            "bm1c": bm1c,
            "w2": W2.astype(bf),
            "bm2r": bm2.reshape(NL, 1, D).astype(bf),
            "wue": np.ascontiguousarray(CU * Wu[:, k * VC:(k + 1) * VC]).astype(f8),
            "bur": np.ascontiguousarray(CL * CU * bu[None, k * VC:(k + 1) * VC]).astype(bf),
        })
    return in_maps


def _run(inputs, **kw):
    if "nc" not in _CACHE:
        _CACHE["nc"] = _build()
    nc = _CACHE["nc"]
    in_maps = _prep_inputs(inputs)
    res = run_bass_kernel_spmd(nc, in_maps, list(range(NC)), **kw)
    outp = np.concatenate([res.results[k]["out"] for k in range(NC)], axis=1)
    return outp.astype(np.float32), res


def kernel(**inputs):
    outp, _ = _run(inputs)
    return outp


# revision 12
# speedup vs baseline: 2.2548x; 1.0760x over previous
"""Trainium2 Bass kernel for the 4-layer dense transformer (nn_DTransformer).

Self-contained: takes full unsharded inputs, shards across 8 NeuronCores
(sequence-parallel residual stream + head-sharded QK-merge + vocab-sharded
unembed), runs one SPMD Bass/Tile kernel, reassembles the full output.

v2: fp8(e4m3) tensor cores for the dominant matmuls (scores via DoubleRow,
QK-merge, MLP up-proj, unembed), SBUF-resident unembed weights with per-row-
tile softmax denominators (no DRAM spill of the exp matrix), halved collective
payloads. Activation scales are folded host-side (see _prep_inputs) and undone
in the activation-function `scale` arguments.
"""
import sys

sys.path.insert(0, "/opt/trn_rl_repo")

import numpy as np
import ml_dtypes

import concourse.bass as bass
import concourse.mybir as mybir
import concourse.tile as tile
from concourse import bacc
from concourse.bass_utils import run_bass_kernel_spmd
from concourse.masks import make_identity

F32 = mybir.dt.float32
BF16 = mybir.dt.bfloat16
F8 = mybir.dt.float8e4
AF = mybir.ActivationFunctionType
ALU = mybir.AluOpType
DR = mybir.MatmulPerfMode.DoubleRow

L, D, H, DV, DM, VOC, NL = 2048, 768, 12, 64, 3072, 32000, 4
NC = 8
R = L // NC            # 256 rows per core
VC = VOC // NC         # 4000 vocab cols per core
ET = D // 128          # 6 feature tiles
JT = DM // 128         # 24 mlp tiles
MT = L // 128          # 16 m (key) tiles
LT = R // 128          # 2 local row tiles
NB = VC // 500         # 8 unembed col blocks
SCALE = float(1.0 / np.sqrt(np.float32(D)))

# fp8 pre-scales (host-side; undone in activation `scale` args)
CL = 4.0               # LN1 / final-LN gamma,beta (lT for scores+unembed)
CW = 8.0               # LN2 gamma,beta (znT for MLP)
CM = 8.0               # each of mq, mk  -> M carries CM*CM = 64
CC = 64.0              # combo v-columns
CCW = 1024.0           # combo w-columns
CU = 64.0              # unembed / MLP-up weight scale

_CACHE = {}


def _pair(t, w, c, lo, hi):
    """[128, 6*w] et-major tile -> [128, 2, hi-lo] fp8 pair view for
    et = (2c, 2c+1), used as a DoubleRow matmul operand."""
    return t[:, 2 * c * w:(2 * c + 2) * w].rearrange("p (e n) -> p e n", e=2)[:, :, lo:hi]


def _build(analyze=False, reps=1):
    nc = bacc.Bacc("TRN2", target_bir_lowering=False, debug=False, num_devices=NC)

    # ---------------- I/O ----------------
    e0 = nc.dram_tensor("e0", [R, D], F32, kind="ExternalInput")
    mq = nc.dram_tensor("mq", [NL, 3, D, D], F8, kind="ExternalInput")
    mk = nc.dram_tensor("mk", [NL, 3, D, 384], F8, kind="ExternalInput")
    lnp = nc.dram_tensor("lnp", [NL, D, 6], F32, kind="ExternalInput")
    lnf = nc.dram_tensor("lnf", [D, 2], F32, kind="ExternalInput")
    combo = nc.dram_tensor("combo", [NL, D, 87], F8, kind="ExternalInput")
    woe = nc.dram_tensor("woe", [NL, 76, D], BF16, kind="ExternalInput")
    w1 = nc.dram_tensor("w1", [NL, D, DM], F8, kind="ExternalInput")
    bm1c = nc.dram_tensor("bm1c", [NL, 128, JT], F32, kind="ExternalInput")
    w2 = nc.dram_tensor("w2", [NL, DM, D], BF16, kind="ExternalInput")
    bm2r = nc.dram_tensor("bm2r", [NL, 1, D], BF16, kind="ExternalInput")
    wue = nc.dram_tensor("wue", [D, VC], F8, kind="ExternalInput")
    bur = nc.dram_tensor("bur", [1, VC], BF16, kind="ExternalInput")
    out = nc.dram_tensor("out", [L, VC], BF16, kind="ExternalOutput")

    # ---------------- internal DRAM ----------------
    xnt_mine = [nc.dram_tensor(f"xnt_mine{i}", [D, R], F8) for i in range(NL + 1)]
    xnt_all = [
        nc.dram_tensor(f"xnt_all{i}", [NC * D, R], F8, addr_space="Shared")
        for i in range(NL + 1)
    ]
    mcontrib = [nc.dram_tensor(f"mcon{i}", [3 * D, 384], F8) for i in range(NL)]
    m_all = [
        nc.dram_tensor(f"mall{i}", [NC * 3 * D, 384], F8, addr_space="Shared")
        for i in range(NL)
    ]
    denm = [nc.dram_tensor(f"denm{g}", [512], F32) for g in range(4)]
    den_allm = [
        nc.dram_tensor(f"den_allm{g}", [512], F32, addr_space="Shared")
        for g in range(4)
    ]

    RG = [list(range(NC))]

    with tile.TileContext(nc) as tc:
      def _one_rep():
            with (
                tc.tile_pool(name="const", bufs=1) as cpool,
                tc.tile_pool(name="pers", bufs=1) as pers,
                tc.tile_pool(name="work", bufs=2) as work,
                tc.tile_pool(name="wt", bufs=1) as wtp,
                tc.tile_pool(name="mqp", bufs=1) as mqp,
                tc.tile_pool(name="mkp", bufs=2) as mkp,
                tc.tile_pool(name="mhp", bufs=3) as mhp,
                tc.tile_pool(name="etp", bufs=4) as etp,
                tc.tile_pool(name="w1p", bufs=6) as w1p,
                tc.tile_pool(name="w2p", bufs=24) as w2p,
                tc.tile_pool(name="gtp", bufs=24) as gtp,
                tc.tile_pool(name="eup", bufs=36) as eup,
                tc.tile_pool(name="scp", bufs=3) as scp,
                tc.tile_pool(name="ps", bufs=2, space="PSUM") as ps,
                tc.tile_pool(name="ps3", bufs=2, space="PSUM") as ps3,
            ):
                # constants
                ident = cpool.tile([128, 128], BF16)
                make_identity(nc, ident[:])
                identf = cpool.tile([128, 128], F32)
                make_identity(nc, identf[:])
                ones_row = cpool.tile([1, 128], BF16)   # K=1 matmul lhsT (all-ones)
                nc.vector.memset(ones_row[:], 1.0)

                # residual stream, f32: Y[:, lt*D + d], row l = lt*128 + p
                Y = pers.tile([128, LT * D], F32)
                for lt in range(LT):
                    nc.sync.dma_start(Y[:, lt * D:(lt + 1) * D], e0[lt * 128:(lt + 1) * 128, :])

                xnTf = pers.tile([128, ET * L], F8)   # gathered feature-major LN out

                def layernorm(pcol_g, pcol_b, raw_cols, want_rowmajor):
                    """LN of Y chunk -> (lT fp8 [128, ET*R] feature-major, zn bf16 or None).

                    pcol_g/pcol_b: et -> sbuf [128, 1] f32 scaled gamma/beta columns.
                    raw_cols: (g_raw, b_raw) accessors for the unscaled row-major copy.
                    """
                    lT = pers.tile([128, ET * R], F8, tag="lT", name="lT")
                    lTb = pers.tile([128, ET * R], BF16, tag="lTb", name="lTb") if want_rowmajor else None
                    zn = pers.tile([128, LT * D], BF16, tag="zn", name="zn") if want_rowmajor else None
                    for lt in range(LT):
                        ys = Y[:, lt * D:(lt + 1) * D]
                        mean = work.tile([128, 1], F32, tag="m1")
                        nc.vector.reduce_sum(mean[:], ys, axis=mybir.AxisListType.X)
                        nmean = work.tile([128, 1], F32, tag="m2")
                        nc.scalar.mul(nmean[:], mean[:], -1.0 / D)
                        cent = work.tile([128, D], BF16, tag="cent")
                        nc.vector.tensor_scalar_add(cent[:], ys, nmean[:])
                        sq = work.tile([128, D], BF16, tag="sq")
                        ssum = work.tile([128, 1], F32, tag="m3")
                        nc.scalar.activation(sq[:], cent[:], AF.Square, accum_out=ssum[:])
                        var = work.tile([128, 1], F32, tag="m4")
                        nc.scalar.mul(var[:], ssum[:], 1.0 / D)
                        std = work.tile([128, 1], F32, tag="m5")
                        nc.scalar.sqrt(std[:], var[:])
                        rstd = work.tile([128, 1], F32, tag="m6")
                        nc.vector.reciprocal(rstd[:], std[:])
                        norm = work.tile([128, D], BF16, tag="norm")
                        nc.vector.tensor_scalar_mul(norm[:], cent[:], rstd[:])
                        for et in range(ET):
                            pt = ps.tile([128, 128], BF16, tag="tr")
                            nc.tensor.transpose(pt[:], norm[:, et * 128:(et + 1) * 128], ident[:])
                            dst = lT[:, et * R + lt * 128: et * R + (lt + 1) * 128]
                            nc.vector.tensor_scalar(
                                dst, pt[:], pcol_g(et), pcol_b(et),
                                op0=ALU.mult, op1=ALU.add,
                            )
                            if want_rowmajor:
                                g_raw, b_raw = raw_cols
                                dstb = lTb[:, et * R + lt * 128: et * R + (lt + 1) * 128]
                                nc.vector.tensor_scalar(
                                    dstb, pt[:], g_raw(et), b_raw(et),
                                    op0=ALU.mult, op1=ALU.add,
                                )
                    if want_rowmajor:
                        for lt in range(LT):
                            for et in range(ET):
                                pt = ps.tile([128, 128], BF16, tag="tr")
                                nc.tensor.transpose(
                                    pt[:], lTb[:, et * R + lt * 128: et * R + (lt + 1) * 128],
                                    ident[:],
                                )
                                nc.vector.tensor_copy(zn[:, lt * D + et * 128: lt * D + (et + 1) * 128], pt[:])
                    return lT, zn

                def gather_lt(lT, mine_dram, all_dram):
                    """DMA local feature-major chunk to DRAM, AllGather, load full."""
                    for et in range(ET):
                        nc.sync.dma_start(
                            mine_dram[et * 128:(et + 1) * 128, :],
                            lT[:, et * R:(et + 1) * R],
                        )
                    if analyze:
                        nc.sync.dma_start(all_dram[0:D, :], mine_dram[:])
                    else:
                        nc.gpsimd.collective_compute(
                            "AllGather", ALU.bypass, replica_groups=RG,
                            ins=[mine_dram[:]], outs=[all_dram[:]],
                        )
                    # all_dram rows: c*D + et*128 + p, cols l_local -> xnTf[p, et*L + c*R + l]
                    v = all_dram[:, :].rearrange("(c e p) l -> e p c l", c=NC, e=ET, p=128)
                    for et in range(ET):
                        dst = xnTf[:, et * L:(et + 1) * L].rearrange("p (c l) -> p c l", c=NC)
                        nc.sync.dma_start(dst, v[et])

                def compute_tT(i, h, hh, tT, lT):
                    mh_sb = []
                    for s in range(2):
                        g = 2 * h + s
                        kcore, p3 = g // 3, g % 3
                        mhs = mhp.tile([128, ET * 384], F8, tag="mh", name="mhs")
                        base = kcore * 3 * D + p3 * D
                        nc.sync.dma_start(
                            mhs[:].rearrange("p (e c) -> p e c", c=384),
                            m_all[i][base:base + D, :].rearrange("(e p) c -> p e c", p=128),
                        )
                        mh_sb.append(mhs)
                    for n6 in range(ET):
                        mhs = mh_sb[n6 // 3]
                        c0 = (n6 % 3) * 128
                        tp = ps3.tile([128, R], F32, tag="smm", name="tp")
                        for et in range(ET):
                            nc.tensor.matmul(
                                tp[:], mhs[:, et * 384 + c0: et * 384 + c0 + 128],
                                lT[:, et * R:(et + 1) * R],
                                start=(et == 0), stop=(et == ET - 1),
                            )
                        nc.vector.tensor_copy(
                            tT[:, n6 * 2 * R + hh * R: n6 * 2 * R + (hh + 1) * R], tp[:]
                        )

                def compute_m(i):
                    for p3 in range(3):
                        mkt = mkp.tile([128, ET * 384], F8, tag="mk", name="mkt")
                        nc.sync.dma_start(
                            mkt[:].rearrange("p (e c) -> p e c", c=384),
                            mk[i, p3].rearrange("(e p) c -> p e c", p=128),
                        )
                        qt = mqp.tile([128, ET * D], F8, tag="mq", name="qt")
                        nc.sync.dma_start(
                            qt[:].rearrange("p (e c) -> p e c", c=D),
                            mq[i, p3].rearrange("(e p) c -> p e c", p=128),
                        )
                        for mt6 in range(ET):
                            mp = ps.tile([128, 384], F32, tag="mm", name="mp")
                            for kc in range(3):
                                nc.tensor.matmul(
                                    mp[:], _pair(qt, D, kc, mt6 * 128, (mt6 + 1) * 128),
                                    _pair(mkt, 384, kc, 0, 384),
                                    start=(kc == 0), stop=(kc == 2), perf_mode=DR,
                                )
                            mo = work.tile([128, 384], F8, tag="mo", name="mo")
                            nc.vector.tensor_copy(mo[:], mp[:])
                            nc.sync.dma_start(
                                mcontrib[i][p3 * D + mt6 * 128: p3 * D + (mt6 + 1) * 128, :],
                                mo[:],
                            )
                    if analyze:
                        nc.sync.dma_start(m_all[i][0:3 * D, :], mcontrib[i][:])
                    else:
                        nc.gpsimd.collective_compute(
                            "AllGather", ALU.bypass, replica_groups=RG,
                            ins=[mcontrib[i][:]], outs=[m_all[i][:]],
                        )

                # layer 0's M first (its AG gates layer 0 attention), then LN1[0]
                # chunk + its AG, then prefetch M for layers 1..3 (their AGs drain
                # while layer 0 computes).
                compute_m(0)

                # ================= layers =================
                for i in range(NL):
                    lnpt = wtp.tile([128, ET * 6], F32, tag="lnp")
                    for et in range(ET):
                        nc.sync.dma_start(
                            lnpt[:, et * 6:(et + 1) * 6], lnp[i, et * 128:(et + 1) * 128, :]
                        )
                    g1c = lambda et: lnpt[:, et * 6 + 0: et * 6 + 1]
                    b1c = lambda et: lnpt[:, et * 6 + 1: et * 6 + 2]
                    g2c = lambda et: lnpt[:, et * 6 + 2: et * 6 + 3]
                    b2c = lambda et: lnpt[:, et * 6 + 3: et * 6 + 4]
                    g2r = lambda et: lnpt[:, et * 6 + 4: et * 6 + 5]
                    b2r = lambda et: lnpt[:, et * 6 + 5: et * 6 + 6]

                    # ---- LN1 -> local feature-major + allgather ----
                    lT, _ = layernorm(g1c, b1c, None, want_rowmajor=False)
                    gather_lt(lT, xnt_mine[i], xnt_all[i])
                    if i + 1 < NL:
                        compute_m(i + 1)

                    # ---- combo: w (12) | v0 (11) | V11 (64) over all m ----
                    cmb = wtp.tile([128, ET * 87], F8, tag="cmb")
                    for et in range(ET):
                        nc.sync.dma_start(
                            cmb[:, et * 87:(et + 1) * 87], combo[i, et * 128:(et + 1) * 128, :]
                        )
                    w_sb = pers.tile([128, MT * 12], F32, tag="wsb")
                    pvl = pers.tile([128, MT * 76], BF16, tag="pvl")
                    for mt in range(MT):
                        cp = ps.tile([128, 87], F32, tag="pv")
                        for et in range(ET):
                            nc.tensor.matmul(
                                cp[:], xnTf[:, et * L + mt * 128: et * L + (mt + 1) * 128],
                                cmb[:, et * 87:(et + 1) * 87],
                                start=(et == 0), stop=(et == ET - 1),
                            )
                        nc.scalar.mul(w_sb[:, mt * 12:(mt + 1) * 12], cp[:, 0:12], 1.0 / (CCW * CL))
                        nc.scalar.mul(pvl[:, mt * 76: mt * 76 + 75], cp[:, 12:87], 1.0 / (CC * CL))
                        nc.vector.memset(pvl[:, mt * 76 + 75: mt * 76 + 76], 1.0)

                    # ---- attention heads ----
                    ylm = pers.tile([128, LT * 76], BF16, tag="ylm")  # l-major y + ones col
                    for lt in range(LT):
                        nc.vector.memset(ylm[:, lt * 76 + 75: lt * 76 + 76], 1.0)
                    woet = wtp.tile([76, D], BF16, tag="woe")
                    nc.sync.dma_start(woet[:], woe[i])

                    for hpair in range(H // 2):
                        # pair heads (2hp, 2hp+1): shared lhsT in the S matmul ->
                        # one N=512 rhs, fp8 DoubleRow halves the instruction count.
                        heads = (2 * hpair, 2 * hpair + 1)
                        tT = work.tile([128, ET * 2 * R], F8, tag="tT", bufs=1, name="tT")
                        for hh, h in enumerate(heads):
                            compute_tT(i, h, hh, tT, lT)
                        pvs = [ps.tile([128, R], F32, tag="pv", name="pv") for _ in range(2)]
                        prev_eTs = None
                        for mt in range(MT):
                            sp = ps3.tile([128, 2 * R], F32, tag="smm", name="sp")
                            for kc in range(3):
                                nc.tensor.matmul(
                                    sp[:], _pair(xnTf, L, kc, mt * 128, (mt + 1) * 128),
                                    _pair(tT, 2 * R, kc, 0, 2 * R),
                                    start=(kc == 0), stop=(kc == 2), perf_mode=DR,
                                )
                            eTs = []
                            for hh, h in enumerate(heads):
                                eT = etp.tile([128, R], BF16, tag="eTm", name="eT")
                                nc.scalar.activation(
                                    eT[:], sp[:, hh * R:(hh + 1) * R], AF.Exp,
                                    bias=w_sb[:, mt * 12 + h: mt * 12 + h + 1],
                                    scale=SCALE / (CL * CL * CM * CM),
                                )
                                eTs.append(eT)
                            # PV delayed one iteration so exp(mt-1) is done when PE gets here
                            if prev_eTs is not None:
                                for hh in range(2):
                                    nc.tensor.matmul(
                                        pvs[hh][0:76, :],
                                        pvl[:, (mt - 1) * 76: mt * 76],
                                        prev_eTs[hh][:],
                                        start=(mt - 1 == 0), stop=False,
                                    )
                            prev_eTs = eTs
                        for hh in range(2):
                            nc.tensor.matmul(
                                pvs[hh][0:76, :],
                                pvl[:, (MT - 1) * 76: MT * 76],
                                prev_eTs[hh][:],
                                start=False, stop=True,
                            )
                        for hh, h in enumerate(heads):
                            pv_sb = work.tile([76, R], F32, tag="pvsb", name="pv_sb")
                            nc.vector.tensor_copy(pv_sb[:], pvs[hh][0:76, :])
                            for lt in range(LT):
                                pvT = ps.tile([128, 76], F32, tag="tr", name="pvT")
                                nc.tensor.transpose(
                                    pvT[:], pv_sb[:, lt * 128:(lt + 1) * 128], identf[0:76, 0:76]
                                )
                                recip = work.tile([128, 1], F32, tag="recip", name="recip")
                                nc.vector.reciprocal(recip[:], pvT[:, 75:76])
                                if h < H - 1:
                                    nc.vector.tensor_scalar_mul(
                                        ylm[:, lt * 76 + h: lt * 76 + h + 1],
                                        pvT[:, h:h + 1], recip[:],
                                    )
                                else:
                                    nc.vector.tensor_scalar_mul(
                                        ylm[:, lt * 76 + 11: lt * 76 + 75],
                                        pvT[:, 11:75], recip[:],
                                    )

                    # ---- out-proj + residual: Y = 2Y + yT.T @ [Wo;bo] ----
                    yT = pers.tile([76, LT * 128], BF16, tag="yT")
                    for lt in range(LT):
                        ytp = ps.tile([128, 128], BF16, tag="tr")
                        nc.tensor.transpose(
                            ytp[0:76, :], ylm[:, lt * 76:(lt + 1) * 76], ident[:]
                        )
                        nc.vector.tensor_copy(yT[:, lt * 128:(lt + 1) * 128], ytp[0:76, :])
                    for lt in range(LT):
                        for nb2 in range(2):
                            ap = ps.tile([128, 384], F32, tag="mm")
                            nc.tensor.matmul(
                                ap[:], yT[:, lt * 128:(lt + 1) * 128],
                                woet[:, nb2 * 384:(nb2 + 1) * 384],
                                start=True, stop=True,
                            )
                            ysl = Y[:, lt * D + nb2 * 384: lt * D + (nb2 + 1) * 384]
                            nc.vector.scalar_tensor_tensor(
                                ysl, ysl, 2.0, ap[:], op0=ALU.mult, op1=ALU.add
                            )

                    # ---- MLP ----
                    znT, zn = layernorm(g2c, b2c, (g2r, b2r), want_rowmajor=True)
                    w1t = [w1p.tile([128, DM], F8, tag="w1", name="w1t") for _ in range(ET)]
                    for et in range(ET):
                        nc.sync.dma_start(w1t[et][:], w1[i, et * 128:(et + 1) * 128, :])
                    bm1t = wtp.tile([128, JT], F32, tag="bm1")
                    nc.sync.dma_start(bm1t[:], bm1c[i])
                    gts = []
                    for jt in range(JT):
                        hp = ps.tile([128, R], F32, tag="mm")
                        for et in range(ET):
                            nc.tensor.matmul(
                                hp[:], w1t[et][:, jt * 128:(jt + 1) * 128],
                                znT[:, et * R:(et + 1) * R],
                                start=(et == 0), stop=(et == ET - 1),
                            )
                        gt = gtp.tile([128, R], BF16, tag="gT")
                        nc.scalar.activation(
                            gt[:], hp[:], AF.Gelu_apprx_tanh,
                            bias=bm1t[:, jt:jt + 1], scale=1.0 / (CW * CU),
                        )
                        gts.append(gt)
                    w2t = [w2p.tile([128, D], BF16, tag="w2", name="w2t") for _ in range(JT)]
                    for jt in range(JT):
                        nc.sync.dma_start(w2t[jt][:], w2[i, jt * 128:(jt + 1) * 128, :])
                    bm2t = wtp.tile([1, D], BF16, tag="bm2")
                    nc.sync.dma_start(bm2t[:], bm2r[i])
                    for lt in range(LT):
                        nc.vector.tensor_add(
                            Y[:, lt * D:(lt + 1) * D], Y[:, lt * D:(lt + 1) * D],
                            zn[:, lt * D:(lt + 1) * D],
                        )
                        for nb2 in range(2):
                            mp2 = ps.tile([128, 384], F32, tag="mm")
                            for jt in range(JT):
                                nc.tensor.matmul(
                                    mp2[:], gts[jt][:, lt * 128:(lt + 1) * 128],
                                    w2t[jt][:, nb2 * 384:(nb2 + 1) * 384],
                                    start=(jt == 0), stop=False,
                                )
                            nc.tensor.matmul(
                                mp2[:], ones_row[:, 0:128],
                                bm2t[:, nb2 * 384:(nb2 + 1) * 384],
                                start=False, stop=True,
                            )
                            ysl = Y[:, lt * D + nb2 * 384: lt * D + (nb2 + 1) * 384]
                            nc.vector.tensor_add(ysl, ysl, mp2[:])

                # ============ final LN + unembed + fused softmax ============
                lnft = wtp.tile([128, ET * 2], F32, tag="lnf")
                for et in range(ET):
                    nc.sync.dma_start(lnft[:, et * 2:(et + 1) * 2], lnf[et * 128:(et + 1) * 128, :])
                gfc = lambda et: lnft[:, et * 2 + 0: et * 2 + 1]
                bfc = lambda et: lnft[:, et * 2 + 1: et * 2 + 2]
                # unembed weights resident in SBUF (fp8), single pass over Wu;
                # issued before the final LN so the loads overlap it
                wu_sb = pers.tile([128, ET * VC], F8, tag="wusb")
                for et in range(ET):
                    nc.sync.dma_start(wu_sb[:, et * VC:(et + 1) * VC], wue[et * 128:(et + 1) * 128, :])
                but = wtp.tile([1, VC], BF16, tag="bu")
                nc.sync.dma_start(but[:], bur[:])

                lT, _ = layernorm(gfc, bfc, None, want_rowmajor=False)
                gather_lt(lT, xnt_mine[NL], xnt_all[NL])

                for grp in range(4):
                    grp_eus = {}
                    for j in range(4):
                        mt = grp * 4 + j
                        dpart = work.tile([128, NB], F32, tag="dpart", name="dpart")
                        eus = []
                        for nb in range(NB):
                            up = ps3.tile([128, 500], F32, tag="smm", name="up")
                            for kc in range(3):
                                nc.tensor.matmul(
                                    up[:], _pair(xnTf, L, kc, mt * 128, (mt + 1) * 128),
                                    _pair(wu_sb, VC, kc, nb * 500, (nb + 1) * 500),
                                    start=(kc == 0), stop=False, perf_mode=DR,
                                )
                            nc.tensor.matmul(
                                up[:], ones_row[:, 0:128], but[:, nb * 500:(nb + 1) * 500],
                                start=False, stop=True,
                            )
                            eu = eup.tile([128, 500], BF16, tag="eu", name="eu")
                            nc.scalar.activation(
                                eu[:], up[:], AF.Exp, scale=1.0 / (CL * CU),
                                accum_out=dpart[:, nb:nb + 1],
                            )
                            eus.append(eu)
                        grp_eus[j] = eus
                        dloc = work.tile([128, 1], F32, tag="dloc", name="dloc")
                        nc.vector.reduce_sum(dloc[:], dpart[:], axis=mybir.AxisListType.X)
                        nc.sync.dma_start(denm[grp][j * 128:(j + 1) * 128], dloc[:])
                    if analyze:
                        nc.sync.dma_start(den_allm[grp][:], denm[grp][:])
                    else:
                        nc.gpsimd.collective_compute(
                            "AllReduce", ALU.add, replica_groups=RG,
                            ins=[denm[grp][:]], outs=[den_allm[grp][:]],
                        )
                    dall = work.tile([128, 4], F32, tag="dall", name="dall")
                    nc.sync.dma_start(dall[:], den_allm[grp][:].rearrange("(m p) -> p m", p=128))
                    drec = work.tile([128, 4], F32, tag="drec", name="drec")
                    nc.vector.reciprocal(drec[:], dall[:])
                    for j in range(4):
                        mt = grp * 4 + j
                        for nb in range(NB):
                            sout = scp.tile([128, 500], BF16, tag="so", name="sout")
                            nc.vector.tensor_scalar_mul(
                                sout[:], grp_eus[j][nb][:], drec[:, j:j + 1]
                            )
                            nc.sync.dma_start(
                                out[mt * 128:(mt + 1) * 128, nb * 500:(nb + 1) * 500], sout[:]
                            )

      for _rep in range(reps):
          _one_rep()

    nc.compile()
    return nc


def _prep_inputs(inputs):
    bf = ml_dtypes.bfloat16
    f8 = mybir.dt.np(F8)
    x = np.asarray(inputs["x"])
    E0 = (np.asarray(inputs["word_embed"])[x] + np.asarray(inputs["pos_embed"])).astype(np.float32)
    Wq, bq = np.asarray(inputs["Wq"]), np.asarray(inputs["bq"])
    Wk = np.asarray(inputs["Wk"])
    Wv, bv = np.asarray(inputs["Wv"]), np.asarray(inputs["bv"])
    Wo, bo = np.asarray(inputs["Wo"]), np.asarray(inputs["bo"])
    W1, bm1 = np.asarray(inputs["W1"]), np.asarray(inputs["bm1"])
    W2, bm2 = np.asarray(inputs["W2"]), np.asarray(inputs["bm2"])
    Wu, bu = np.asarray(inputs["Wu"]), np.asarray(inputs["bu"])

    g1 = np.asarray(inputs["g1"]); be1 = np.asarray(inputs["be1"])
    g2 = np.asarray(inputs["g2"]); be2 = np.asarray(inputs["be2"])
    lnp = np.stack(
        [CL * g1, CL * be1, CW * g2, CW * be2, g2, be2], axis=-1
    ).astype(np.float32)                                   # [NL, D, 6]
    lnf = np.stack(
        [CL * np.asarray(inputs["gf"]), CL * np.asarray(inputs["bef"])], -1
    ).astype(np.float32)

    combo = np.zeros((NL, D, 87), np.float32)
    for i in range(NL):
        for h in range(H):
            combo[i, :, h] = (Wk[i, h] @ bq[i, h]) * SCALE * CCW  # u_scaled
        combo[i, :, 12:23] = Wv[i, :11, :, 0].transpose(1, 0) * CC
        combo[i, :, 23:87] = Wv[i, 11] * CC
    woe = np.zeros((NL, 76, D), np.float32)
    for i in range(NL):
        bv_flat = np.concatenate([bv[i, :11, 0], bv[i, 11]])
        woe[i, :75] = Wo[i, :75]
        woe[i, 75] = bo[i] + bv_flat @ Wo[i, :75]
    bm1c = bm1.reshape(NL, JT, 128).transpose(0, 2, 1).astype(np.float32)

    in_maps = []
    for k in range(NC):
        mqk = np.zeros((NL, 3, D, D), np.float32)
        mkk = np.zeros((NL, 3, D, 384), np.float32)
        for p3 in range(3):
            g = 3 * k + p3
            h, s = g // 2, g % 2
            for i in range(NL):
                mqk[i, p3] = CM * Wq[i, h].T
                mkk[i, p3] = CM * Wk[i, h][s * 384:(s + 1) * 384, :].T
        in_maps.append({
            "e0": E0[k * R:(k + 1) * R],
            "mq": mqk.astype(f8),
            "mk": mkk.astype(f8),
            "lnp": lnp,
            "lnf": lnf,
            "combo": combo.astype(f8),
            "woe": woe.astype(bf),
            "w1": (CU * W1).astype(f8),
            "bm1c": bm1c,
            "w2": W2.astype(bf),
            "bm2r": bm2.reshape(NL, 1, D).astype(bf),
            "wue": np.ascontiguousarray(CU * Wu[:, k * VC:(k + 1) * VC]).astype(f8),
            "bur": np.ascontiguousarray(CL * CU * bu[None, k * VC:(k + 1) * VC]).astype(bf),
        })
    return in_maps


def _run(inputs, **kw):
    if "nc" not in _CACHE:
        _CACHE["nc"] = _build()
    nc = _CACHE["nc"]
    in_maps = _prep_inputs(inputs)
    res = run_bass_kernel_spmd(nc, in_maps, list(range(NC)), **kw)
    outp = np.concatenate([res.results[k]["out"] for k in range(NC)], axis=1)
    return outp.astype(np.float32), res


def kernel(**inputs):
    outp, _ = _run(inputs)
    return outp


# revision 14
# speedup vs baseline: 2.2741x; 1.0086x over previous
"""Trainium2 Bass kernel for the 4-layer dense transformer (nn_DTransformer).

Self-contained: takes full unsharded inputs, shards across 8 NeuronCores
(sequence-parallel residual stream + head-sharded QK-merge + vocab-sharded
unembed), runs one SPMD Bass/Tile kernel, reassembles the full output.

v2: fp8(e4m3) tensor cores for the dominant matmuls (scores via DoubleRow,
QK-merge, MLP up-proj, unembed), SBUF-resident unembed weights with per-row-
tile softmax denominators (no DRAM spill of the exp matrix), halved collective
payloads. Activation scales are folded host-side (see _prep_inputs) and undone
in the activation-function `scale` arguments.
"""
import sys

sys.path.insert(0, "/opt/trn_rl_repo")

import numpy as np
import ml_dtypes

import concourse.bass as bass
import concourse.mybir as mybir
import concourse.tile as tile
from concourse import bacc
from concourse.bass_utils import run_bass_kernel_spmd
from concourse.masks import make_identity

F32 = mybir.dt.float32
BF16 = mybir.dt.bfloat16
F8 = mybir.dt.float8e4
AF = mybir.ActivationFunctionType
ALU = mybir.AluOpType
DR = mybir.MatmulPerfMode.DoubleRow

L, D, H, DV, DM, VOC, NL = 2048, 768, 12, 64, 3072, 32000, 4
NC = 8
R = L // NC            # 256 rows per core
VC = VOC // NC         # 4000 vocab cols per core
ET = D // 128          # 6 feature tiles
JT = DM // 128         # 24 mlp tiles
MT = L // 128          # 16 m (key) tiles
LT = R // 128          # 2 local row tiles
NB = VC // 500         # 8 unembed col blocks
SCALE = float(1.0 / np.sqrt(np.float32(D)))

# fp8 pre-scales (host-side; undone in activation `scale` args)
CL = 4.0               # LN1 / final-LN gamma,beta (lT for scores+unembed)
CW = 8.0               # LN2 gamma,beta (znT for MLP)
CM = 8.0               # each of mq, mk  -> M carries CM*CM = 64
CC = 64.0              # combo v-columns
CCW = 1024.0           # combo w-columns
CU = 64.0              # unembed / MLP-up weight scale

_CACHE = {}


def _pair(t, w, c, lo, hi):
    """[128, 6*w] et-major tile -> [128, 2, hi-lo] fp8 pair view for
    et = (2c, 2c+1), used as a DoubleRow matmul operand."""
    return t[:, 2 * c * w:(2 * c + 2) * w].rearrange("p (e n) -> p e n", e=2)[:, :, lo:hi]


def _build(analyze=False, reps=1):
    nc = bacc.Bacc("TRN2", target_bir_lowering=False, debug=False, num_devices=NC)

    # ---------------- I/O ----------------
    e0 = nc.dram_tensor("e0", [R, D], F32, kind="ExternalInput")
    mq = nc.dram_tensor("mq", [NL, 3, D, D], F8, kind="ExternalInput")
    mk = nc.dram_tensor("mk", [NL, 3, D, 384], F8, kind="ExternalInput")
    lnp = nc.dram_tensor("lnp", [NL, D, 6], F32, kind="ExternalInput")
    lnf = nc.dram_tensor("lnf", [D, 2], F32, kind="ExternalInput")
    combo = nc.dram_tensor("combo", [NL, D, 87], F8, kind="ExternalInput")
    woe = nc.dram_tensor("woe", [NL, 76, D], BF16, kind="ExternalInput")
    w1 = nc.dram_tensor("w1", [NL, D, DM], F8, kind="ExternalInput")
    bm1c = nc.dram_tensor("bm1c", [NL, 128, JT], F32, kind="ExternalInput")
    w2 = nc.dram_tensor("w2", [NL, DM, D], BF16, kind="ExternalInput")
    bm2r = nc.dram_tensor("bm2r", [NL, 1, D], BF16, kind="ExternalInput")
    wue = nc.dram_tensor("wue", [D, VC], F8, kind="ExternalInput")
    bur = nc.dram_tensor("bur", [1, VC], BF16, kind="ExternalInput")
    out = nc.dram_tensor("out", [L, VC], BF16, kind="ExternalOutput")

    # ---------------- internal DRAM ----------------
    xnt_mine = [nc.dram_tensor(f"xnt_mine{i}", [D, R], F8) for i in range(NL + 1)]
    xnt_all = [
        nc.dram_tensor(f"xnt_all{i}", [NC * D, R], F8, addr_space="Shared")
        for i in range(NL + 1)
    ]
    mcontrib = [nc.dram_tensor(f"mcon{i}", [3 * D, 384], F8) for i in range(NL)]
    m_all = [
        nc.dram_tensor(f"mall{i}", [NC * 3 * D, 384], F8, addr_space="Shared")
        for i in range(NL)
    ]
    denm = [nc.dram_tensor(f"denm{g}", [512], F32) for g in range(4)]
    den_allm = [
        nc.dram_tensor(f"den_allm{g}", [512], F32, addr_space="Shared")
        for g in range(4)
    ]

    RG = [list(range(NC))]

    with tile.TileContext(nc) as tc:
      with (
                tc.tile_pool(name="const", bufs=1) as cpool,
                tc.tile_pool(name="pers", bufs=1) as pers,
                tc.tile_pool(name="work", bufs=2) as work,
                tc.tile_pool(name="wt", bufs=1) as wtp,
                tc.tile_pool(name="mqp", bufs=1) as mqp,
                tc.tile_pool(name="mkp", bufs=2) as mkp,
                tc.tile_pool(name="mhp", bufs=3) as mhp,
                tc.tile_pool(name="etp", bufs=4) as etp,
                tc.tile_pool(name="w1p", bufs=6) as w1p,
                tc.tile_pool(name="w2p", bufs=24) as w2p,
                tc.tile_pool(name="gtp", bufs=24) as gtp,
                tc.tile_pool(name="eup", bufs=36) as eup,
                tc.tile_pool(name="scp", bufs=3) as scp,
                tc.tile_pool(name="ps", bufs=2, space="PSUM") as ps,
                tc.tile_pool(name="ps3", bufs=2, space="PSUM") as ps3,
      ):
        def _one_rep():
                # constants
                ident = cpool.tile([128, 128], BF16)
                make_identity(nc, ident[:])
                identf = cpool.tile([128, 128], F32)
                make_identity(nc, identf[:])
                ones_row = cpool.tile([1, 128], BF16)   # K=1 matmul lhsT (all-ones)
                nc.vector.memset(ones_row[:], 1.0)

                # residual stream, f32: Y[:, lt*D + d], row l = lt*128 + p
                Y = pers.tile([128, LT * D], F32)
                for lt in range(LT):
                    nc.sync.dma_start(Y[:, lt * D:(lt + 1) * D], e0[lt * 128:(lt + 1) * 128, :])

                xnTf = pers.tile([128, ET * L], F8)   # gathered feature-major LN out

                def layernorm(pcol_g, pcol_b, raw_cols, want_rowmajor):
                    """LN of Y chunk -> (lT fp8 [128, ET*R] feature-major, zn bf16 or None).

                    pcol_g/pcol_b: et -> sbuf [128, 1] f32 scaled gamma/beta columns.
                    raw_cols: (g_raw, b_raw) accessors for the unscaled row-major copy.
                    """
                    lT = pers.tile([128, ET * R], F8, tag="lT", name="lT")
                    lTb = pers.tile([128, ET * R], BF16, tag="lTb", name="lTb") if want_rowmajor else None
                    zn = pers.tile([128, LT * D], BF16, tag="zn", name="zn") if want_rowmajor else None
                    for lt in range(LT):
                        ys = Y[:, lt * D:(lt + 1) * D]
                        mean = work.tile([128, 1], F32, tag="m1")
                        nc.vector.reduce_sum(mean[:], ys, axis=mybir.AxisListType.X)
                        nmean = work.tile([128, 1], F32, tag="m2")
                        nc.scalar.mul(nmean[:], mean[:], -1.0 / D)
                        cent = work.tile([128, D], BF16, tag="cent")
                        nc.vector.tensor_scalar_add(cent[:], ys, nmean[:])
                        sq = work.tile([128, D], BF16, tag="sq")
                        ssum = work.tile([128, 1], F32, tag="m3")
                        nc.scalar.activation(sq[:], cent[:], AF.Square, accum_out=ssum[:])
                        var = work.tile([128, 1], F32, tag="m4")
                        nc.scalar.mul(var[:], ssum[:], 1.0 / D)
                        std = work.tile([128, 1], F32, tag="m5")
                        nc.scalar.sqrt(std[:], var[:])
                        rstd = work.tile([128, 1], F32, tag="m6")
                        nc.vector.reciprocal(rstd[:], std[:])
                        norm = work.tile([128, D], BF16, tag="norm")
                        nc.vector.tensor_scalar_mul(norm[:], cent[:], rstd[:])
                        for et in range(ET):
                            pt = ps.tile([128, 128], BF16, tag="tr")
                            nc.tensor.transpose(pt[:], norm[:, et * 128:(et + 1) * 128], ident[:])
                            dst = lT[:, et * R + lt * 128: et * R + (lt + 1) * 128]
                            nc.vector.tensor_scalar(
                                dst, pt[:], pcol_g(et), pcol_b(et),
                                op0=ALU.mult, op1=ALU.add,
                            )
                            if want_rowmajor:
                                g_raw, b_raw = raw_cols
                                dstb = lTb[:, et * R + lt * 128: et * R + (lt + 1) * 128]
                                nc.vector.tensor_scalar(
                                    dstb, pt[:], g_raw(et), b_raw(et),
                                    op0=ALU.mult, op1=ALU.add,
                                )
                    if want_rowmajor:
                        for lt in range(LT):
                            for et in range(ET):
                                pt = ps.tile([128, 128], BF16, tag="tr")
                                nc.tensor.transpose(
                                    pt[:], lTb[:, et * R + lt * 128: et * R + (lt + 1) * 128],
                                    ident[:],
                                )
                                nc.vector.tensor_copy(zn[:, lt * D + et * 128: lt * D + (et + 1) * 128], pt[:])
                    return lT, zn

                def gather_lt(lT, mine_dram, all_dram):
                    """DMA local feature-major chunk to DRAM, AllGather, load full."""
                    for et in range(ET):
                        nc.sync.dma_start(
                            mine_dram[et * 128:(et + 1) * 128, :],
                            lT[:, et * R:(et + 1) * R],
                        )
                    if analyze:
                        nc.sync.dma_start(all_dram[0:D, :], mine_dram[:])
                    else:
                        nc.gpsimd.collective_compute(
                            "AllGather", ALU.bypass, replica_groups=RG,
                            ins=[mine_dram[:]], outs=[all_dram[:]],
                        )
                    # all_dram rows: c*D + et*128 + p, cols l_local -> xnTf[p, et*L + c*R + l]
                    v = all_dram[:, :].rearrange("(c e p) l -> e p c l", c=NC, e=ET, p=128)
                    for et in range(ET):
                        dst = xnTf[:, et * L:(et + 1) * L].rearrange("p (c l) -> p c l", c=NC)
                        nc.sync.dma_start(dst, v[et])

                def compute_tT(i, h, hh, tT, lT):
                    mh_sb = []
                    for s in range(2):
                        g = 2 * h + s
                        kcore, p3 = g // 3, g % 3
                        mhs = mhp.tile([128, ET * 384], F8, tag="mh", name="mhs")
                        base = kcore * 3 * D + p3 * D
                        nc.sync.dma_start(
                            mhs[:].rearrange("p (e c) -> p e c", c=384),
                            m_all[i][base:base + D, :].rearrange("(e p) c -> p e c", p=128),
                        )
                        mh_sb.append(mhs)
                    for n6 in range(ET):
                        mhs = mh_sb[n6 // 3]
                        c0 = (n6 % 3) * 128
                        tp = ps3.tile([128, R], F32, tag="smm", name="tp")
                        for et in range(ET):
                            nc.tensor.matmul(
                                tp[:], mhs[:, et * 384 + c0: et * 384 + c0 + 128],
                                lT[:, et * R:(et + 1) * R],
                                start=(et == 0), stop=(et == ET - 1),
                            )
                        nc.vector.tensor_copy(
                            tT[:, n6 * 2 * R + hh * R: n6 * 2 * R + (hh + 1) * R], tp[:]
                        )

                def compute_m(i):
                    for p3 in range(3):
                        mkt = mkp.tile([128, ET * 384], F8, tag="mk", name="mkt")
                        nc.sync.dma_start(
                            mkt[:].rearrange("p (e c) -> p e c", c=384),
                            mk[i, p3].rearrange("(e p) c -> p e c", p=128),
                        )
                        qt = mqp.tile([128, ET * D], F8, tag="mq", name="qt")
                        nc.sync.dma_start(
                            qt[:].rearrange("p (e c) -> p e c", c=D),
                            mq[i, p3].rearrange("(e p) c -> p e c", p=128),
                        )
                        for mt6 in range(ET):
                            mp = ps.tile([128, 384], F32, tag="mm", name="mp")
                            for kc in range(3):
                                nc.tensor.matmul(
                                    mp[:], _pair(qt, D, kc, mt6 * 128, (mt6 + 1) * 128),
                                    _pair(mkt, 384, kc, 0, 384),
                                    start=(kc == 0), stop=(kc == 2), perf_mode=DR,
                                )
                            mo = work.tile([128, 384], F8, tag="mo", name="mo")
                            nc.vector.tensor_copy(mo[:], mp[:])
                            nc.sync.dma_start(
                                mcontrib[i][p3 * D + mt6 * 128: p3 * D + (mt6 + 1) * 128, :],
                                mo[:],
                            )
                    if analyze:
                        nc.sync.dma_start(m_all[i][0:3 * D, :], mcontrib[i][:])
                    else:
                        nc.gpsimd.collective_compute(
                            "AllGather", ALU.bypass, replica_groups=RG,
                            ins=[mcontrib[i][:]], outs=[m_all[i][:]],
                        )

                # layer 0's M first (its AG gates layer 0 attention), then LN1[0]
                # chunk + its AG, then prefetch M for layers 1..3 (their AGs drain
                # while layer 0 computes).
                compute_m(0)

                # ================= layers =================
                for i in range(NL):
                    lnpt = wtp.tile([128, ET * 6], F32, tag="lnp")
                    for et in range(ET):
                        nc.sync.dma_start(
                            lnpt[:, et * 6:(et + 1) * 6], lnp[i, et * 128:(et + 1) * 128, :]
                        )
                    g1c = lambda et: lnpt[:, et * 6 + 0: et * 6 + 1]
                    b1c = lambda et: lnpt[:, et * 6 + 1: et * 6 + 2]
                    g2c = lambda et: lnpt[:, et * 6 + 2: et * 6 + 3]
                    b2c = lambda et: lnpt[:, et * 6 + 3: et * 6 + 4]
                    g2r = lambda et: lnpt[:, et * 6 + 4: et * 6 + 5]
                    b2r = lambda et: lnpt[:, et * 6 + 5: et * 6 + 6]

                    # ---- LN1 -> local feature-major + allgather ----
                    lT, _ = layernorm(g1c, b1c, None, want_rowmajor=False)
                    gather_lt(lT, xnt_mine[i], xnt_all[i])
                    if i + 1 < NL:
                        compute_m(i + 1)

                    # ---- combo: w (12) | v0 (11) | V11 (64) over all m ----
                    cmb = wtp.tile([128, ET * 87], F8, tag="cmb")
                    for et in range(ET):
                        nc.sync.dma_start(
                            cmb[:, et * 87:(et + 1) * 87], combo[i, et * 128:(et + 1) * 128, :]
                        )
                    w_sb = pers.tile([128, MT * 12], F32, tag="wsb")
                    pvl = pers.tile([128, MT * 80], F8, tag="pvl")  # stride 80: dual-fp8 LDW needs step%16==0
                    for mt in range(MT):
                        cp = ps.tile([128, 87], F32, tag="pv")
                        for et in range(ET):
                            nc.tensor.matmul(
                                cp[:], xnTf[:, et * L + mt * 128: et * L + (mt + 1) * 128],
                                cmb[:, et * 87:(et + 1) * 87],
                                start=(et == 0), stop=(et == ET - 1),
                            )
                        nc.scalar.mul(w_sb[:, mt * 12:(mt + 1) * 12], cp[:, 0:12], 1.0 / (CCW * CL))
                        nc.scalar.mul(pvl[:, mt * 80: mt * 80 + 75], cp[:, 12:87], 0.5)
                        nc.vector.memset(pvl[:, mt * 80 + 75: mt * 80 + 76], 128.0)

                    # ---- attention heads ----
                    ylm = pers.tile([128, LT * 76], BF16, tag="ylm")  # l-major y + ones col
                    for lt in range(LT):
                        nc.vector.memset(ylm[:, lt * 76 + 75: lt * 76 + 76], 1.0)
                    woet = wtp.tile([76, D], BF16, tag="woe")
                    nc.sync.dma_start(woet[:], woe[i])

                    for hpair in range(H // 2):
                        # pair heads (2hp, 2hp+1): shared lhsT in the S matmul ->
                        # one N=512 rhs, fp8 DoubleRow halves the instruction count.
                        heads = (2 * hpair, 2 * hpair + 1)
                        tT = work.tile([128, ET * 2 * R], F8, tag="tT", bufs=1, name="tT")
                        for hh, h in enumerate(heads):
                            compute_tT(i, h, hh, tT, lT)
                        pvs = [ps.tile([128, R], F32, tag="pv", name="pv") for _ in range(2)]
                        prev_eTs = None
                        for mp in range(MT // 2):
                            eT2s = [etp.tile([128, 2 * R], F8, tag="eTm", name="eT2")
                                    for _ in range(2)]
                            for sub in range(2):
                                mt = 2 * mp + sub
                                sp = ps3.tile([128, 2 * R], F32, tag="smm", name="sp")
                                for kc in range(3):
                                    nc.tensor.matmul(
                                        sp[:], _pair(xnTf, L, kc, mt * 128, (mt + 1) * 128),
                                        _pair(tT, 2 * R, kc, 0, 2 * R),
                                        start=(kc == 0), stop=(kc == 2), perf_mode=DR,
                                    )
                                for hh, h in enumerate(heads):
                                    nc.scalar.activation(
                                        eT2s[hh][:, sub * R:(sub + 1) * R],
                                        sp[:, hh * R:(hh + 1) * R], AF.Exp,
                                        bias=w_sb[:, mt * 12 + h: mt * 12 + h + 1],
                                        scale=SCALE / (CL * CL * CM * CM),
                                    )
                            # PV (fp8 DR over 256-key pairs) delayed one mt-pair so
                            # the exps are done when PE gets here
                            if prev_eTs is not None:
                                for hh in range(2):
                                    nc.tensor.matmul(
                                        pvs[hh][0:76, :],
                                        _pair(pvl, 80, mp - 1, 0, 76),
                                        prev_eTs[hh][:].rearrange("p (e n) -> p e n", e=2),
                                        start=(mp - 1 == 0), stop=False, perf_mode=DR,
                                    )
                            prev_eTs = eT2s
                        for hh in range(2):
                            nc.tensor.matmul(
                                pvs[hh][0:76, :],
                                _pair(pvl, 80, MT // 2 - 1, 0, 76),
                                prev_eTs[hh][:].rearrange("p (e n) -> p e n", e=2),
                                start=False, stop=True, perf_mode=DR,
                            )
                        for hh, h in enumerate(heads):
                            pv_sb = work.tile([76, R], F32, tag="pvsb", name="pv_sb")
                            nc.vector.tensor_copy(pv_sb[:], pvs[hh][0:76, :])
                            for lt in range(LT):
                                pvT = ps.tile([128, 76], F32, tag="tr", name="pvT")
                                nc.tensor.transpose(
                                    pvT[:], pv_sb[:, lt * 128:(lt + 1) * 128], identf[0:76, 0:76]
                                )
                                recip = work.tile([128, 1], F32, tag="recip", name="recip")
                                nc.vector.reciprocal(recip[:], pvT[:, 75:76])
                                if h < H - 1:
                                    nc.vector.tensor_scalar_mul(
                                        ylm[:, lt * 76 + h: lt * 76 + h + 1],
                                        pvT[:, h:h + 1], recip[:],
                                    )
                                else:
                                    nc.vector.tensor_scalar_mul(
                                        ylm[:, lt * 76 + 11: lt * 76 + 75],
                                        pvT[:, 11:75], recip[:],
                                    )

                    # ---- out-proj + residual: Y = 2Y + yT.T @ [Wo;bo] ----
                    yT = pers.tile([76, LT * 128], BF16, tag="yT")
                    for lt in range(LT):
                        ytp = ps.tile([128, 128], BF16, tag="tr")
                        nc.tensor.transpose(
                            ytp[0:76, :], ylm[:, lt * 76:(lt + 1) * 76], ident[:]
                        )
                        nc.vector.tensor_copy(yT[:, lt * 128:(lt + 1) * 128], ytp[0:76, :])
                    for lt in range(LT):
                        for nb2 in range(2):
                            ap = ps.tile([128, 384], F32, tag="mm")
                            nc.tensor.matmul(
                                ap[:], yT[:, lt * 128:(lt + 1) * 128],
                                woet[:, nb2 * 384:(nb2 + 1) * 384],
                                start=True, stop=True,
                            )
                            ysl = Y[:, lt * D + nb2 * 384: lt * D + (nb2 + 1) * 384]
                            nc.vector.scalar_tensor_tensor(
                                ysl, ysl, 2.0, ap[:], op0=ALU.mult, op1=ALU.add
                            )

                    # ---- MLP ----
                    znT, zn = layernorm(g2c, b2c, (g2r, b2r), want_rowmajor=True)
                    w1t = [w1p.tile([128, DM], F8, tag="w1", name="w1t") for _ in range(ET)]
                    for et in range(ET):
                        nc.sync.dma_start(w1t[et][:], w1[i, et * 128:(et + 1) * 128, :])
                    bm1t = wtp.tile([128, JT], F32, tag="bm1")
                    nc.sync.dma_start(bm1t[:], bm1c[i])
                    gts = []
                    for jt in range(JT):
                        hp = ps.tile([128, R], F32, tag="mm")
                        for et in range(ET):
                            nc.tensor.matmul(
                                hp[:], w1t[et][:, jt * 128:(jt + 1) * 128],
                                znT[:, et * R:(et + 1) * R],
                                start=(et == 0), stop=(et == ET - 1),
                            )
                        gt = gtp.tile([128, R], BF16, tag="gT")
                        nc.scalar.activation(
                            gt[:], hp[:], AF.Gelu_apprx_tanh,
                            bias=bm1t[:, jt:jt + 1], scale=1.0 / (CW * CU),
                        )
                        gts.append(gt)
                    w2t = [w2p.tile([128, D], BF16, tag="w2", name="w2t") for _ in range(JT)]
                    for jt in range(JT):
                        nc.sync.dma_start(w2t[jt][:], w2[i, jt * 128:(jt + 1) * 128, :])
                    bm2t = wtp.tile([1, D], BF16, tag="bm2")
                    nc.sync.dma_start(bm2t[:], bm2r[i])
                    for lt in range(LT):
                        nc.vector.tensor_add(
                            Y[:, lt * D:(lt + 1) * D], Y[:, lt * D:(lt + 1) * D],
                            zn[:, lt * D:(lt + 1) * D],
                        )
                        for nb2 in range(2):
                            mp2 = ps.tile([128, 384], F32, tag="mm")
                            for jt in range(JT):
                                nc.tensor.matmul(
                                    mp2[:], gts[jt][:, lt * 128:(lt + 1) * 128],
                                    w2t[jt][:, nb2 * 384:(nb2 + 1) * 384],
                                    start=(jt == 0), stop=False,
                                )
                            nc.tensor.matmul(
                                mp2[:], ones_row[:, 0:128],
                                bm2t[:, nb2 * 384:(nb2 + 1) * 384],
                                start=False, stop=True,
                            )
                            ysl = Y[:, lt * D + nb2 * 384: lt * D + (nb2 + 1) * 384]
                            nc.vector.tensor_add(ysl, ysl, mp2[:])

                # ============ final LN + unembed + fused softmax ============
                lnft = wtp.tile([128, ET * 2], F32, tag="lnf")
                for et in range(ET):
                    nc.sync.dma_start(lnft[:, et * 2:(et + 1) * 2], lnf[et * 128:(et + 1) * 128, :])
                gfc = lambda et: lnft[:, et * 2 + 0: et * 2 + 1]
                bfc = lambda et: lnft[:, et * 2 + 1: et * 2 + 2]
                # unembed weights resident in SBUF (fp8), single pass over Wu;
                # issued before the final LN so the loads overlap it
                wu_sb = pers.tile([128, ET * VC], F8, tag="wusb")
                for et in range(ET):
                    nc.sync.dma_start(wu_sb[:, et * VC:(et + 1) * VC], wue[et * 128:(et + 1) * 128, :])
                but = wtp.tile([1, VC], BF16, tag="bu")
                nc.sync.dma_start(but[:], bur[:])

                lT, _ = layernorm(gfc, bfc, None, want_rowmajor=False)
                gather_lt(lT, xnt_mine[NL], xnt_all[NL])

                for grp in range(4):
                    grp_eus = {}
                    for j in range(4):
                        mt = grp * 4 + j
                        dpart = work.tile([128, NB], F32, tag="dpart", name="dpart")
                        eus = []
                        for nb in range(NB):
                            up = ps3.tile([128, 500], F32, tag="smm", name="up")
                            for kc in range(3):
                                nc.tensor.matmul(
                                    up[:], _pair(xnTf, L, kc, mt * 128, (mt + 1) * 128),
                                    _pair(wu_sb, VC, kc, nb * 500, (nb + 1) * 500),
                                    start=(kc == 0), stop=False, perf_mode=DR,
                                )
                            nc.tensor.matmul(
                                up[:], ones_row[:, 0:128], but[:, nb * 500:(nb + 1) * 500],
                                start=False, stop=True,
                            )
                            eu = eup.tile([128, 500], BF16, tag="eu", name="eu")
                            nc.scalar.activation(
                                eu[:], up[:], AF.Exp, scale=1.0 / (CL * CU),
                                accum_out=dpart[:, nb:nb + 1],
                            )
                            eus.append(eu)
                        grp_eus[j] = eus
                        dloc = work.tile([128, 1], F32, tag="dloc", name="dloc")
                        nc.vector.reduce_sum(dloc[:], dpart[:], axis=mybir.AxisListType.X)
                        nc.sync.dma_start(denm[grp][j * 128:(j + 1) * 128], dloc[:])
                    if analyze:
                        nc.sync.dma_start(den_allm[grp][:], denm[grp][:])
                    else:
                        nc.gpsimd.collective_compute(
                            "AllReduce", ALU.add, replica_groups=RG,
                            ins=[denm[grp][:]], outs=[den_allm[grp][:]],
                        )
                    dall = work.tile([128, 4], F32, tag="dall", name="dall")
                    nc.sync.dma_start(dall[:], den_allm[grp][:].rearrange("(m p) -> p m", p=128))
                    drec = work.tile([128, 4], F32, tag="drec", name="drec")
                    nc.vector.reciprocal(drec[:], dall[:])
                    for j in range(4):
                        mt = grp * 4 + j
                        for nb in range(NB):
                            sout = scp.tile([128, 500], BF16, tag="so", name="sout")
                            nc.vector.tensor_scalar_mul(
                                sout[:], grp_eus[j][nb][:], drec[:, j:j + 1]
                            )
                            nc.sync.dma_start(
                                out[mt * 128:(mt + 1) * 128, nb * 500:(nb + 1) * 500], sout[:]
                            )

        for _rep in range(reps):
            _one_rep()

    nc.compile()
    return nc


def _prep_inputs(inputs):
    bf = ml_dtypes.bfloat16
    f8 = mybir.dt.np(F8)
    x = np.asarray(inputs["x"])
    E0 = (np.asarray(inputs["word_embed"])[x] + np.asarray(inputs["pos_embed"])).astype(np.float32)
    Wq, bq = np.asarray(inputs["Wq"]), np.asarray(inputs["bq"])
    Wk = np.asarray(inputs["Wk"])
    Wv, bv = np.asarray(inputs["Wv"]), np.asarray(inputs["bv"])
    Wo, bo = np.asarray(inputs["Wo"]), np.asarray(inputs["bo"])
    W1, bm1 = np.asarray(inputs["W1"]), np.asarray(inputs["bm1"])
    W2, bm2 = np.asarray(inputs["W2"]), np.asarray(inputs["bm2"])
    Wu, bu = np.asarray(inputs["Wu"]), np.asarray(inputs["bu"])

    g1 = np.asarray(inputs["g1"]); be1 = np.asarray(inputs["be1"])
    g2 = np.asarray(inputs["g2"]); be2 = np.asarray(inputs["be2"])
    lnp = np.stack(
        [CL * g1, CL * be1, CW * g2, CW * be2, g2, be2], axis=-1
    ).astype(np.float32)                                   # [NL, D, 6]
    lnf = np.stack(
        [CL * np.asarray(inputs["gf"]), CL * np.asarray(inputs["bef"])], -1
    ).astype(np.float32)

    combo = np.zeros((NL, D, 87), np.float32)
    for i in range(NL):
        for h in range(H):
            combo[i, :, h] = (Wk[i, h] @ bq[i, h]) * SCALE * CCW  # u_scaled
        combo[i, :, 12:23] = Wv[i, :11, :, 0].transpose(1, 0) * CC
        combo[i, :, 23:87] = Wv[i, 11] * CC
    woe = np.zeros((NL, 76, D), np.float32)
    for i in range(NL):
        bv_flat = np.concatenate([bv[i, :11, 0], bv[i, 11]])
        woe[i, :75] = Wo[i, :75]
        woe[i, 75] = bo[i] + bv_flat @ Wo[i, :75]
    bm1c = bm1.reshape(NL, JT, 128).transpose(0, 2, 1).astype(np.float32)

    in_maps = []
    for k in range(NC):
        mqk = np.zeros((NL, 3, D, D), np.float32)
        mkk = np.zeros((NL, 3, D, 384), np.float32)
        for p3 in range(3):
            g = 3 * k + p3
            h, s = g // 2, g % 2
            for i in range(NL):
                mqk[i, p3] = CM * Wq[i, h].T
                mkk[i, p3] = CM * Wk[i, h][s * 384:(s + 1) * 384, :].T
        in_maps.append({
            "e0": E0[k * R:(k + 1) * R],
            "mq": mqk.astype(f8),
            "mk": mkk.astype(f8),
            "lnp": lnp,
            "lnf": lnf,
            "combo": combo.astype(f8),
            "woe": woe.astype(bf),
            "w1": (CU * W1).astype(f8),
            "bm1c": bm1c,
            "w2": W2.astype(bf),
            "bm2r": bm2.reshape(NL, 1, D).astype(bf),
            "wue": np.ascontiguousarray(CU * Wu[:, k * VC:(k + 1) * VC]).astype(f8),
            "bur": np.ascontiguousarray(CL * CU * bu[None, k * VC:(k + 1) * VC]).astype(bf),
        })
    return in_maps


def _run(inputs, **kw):
    if "nc" not in _CACHE:
        _CACHE["nc"] = _build()
    nc = _CACHE["nc"]
    in_maps = _prep_inputs(inputs)
    res = run_bass_kernel_spmd(nc, in_maps, list(range(NC)), **kw)
    outp = np.concatenate([res.results[k]["out"] for k in range(NC)], axis=1)
    return outp.astype(np.float32), res


def kernel(**inputs):
    outp, _ = _run(inputs)
    return outp


# revision 15
# speedup vs baseline: 2.3180x; 1.0193x over previous
"""Trainium2 Bass kernel for the 4-layer dense transformer (nn_DTransformer).

Self-contained: takes full unsharded inputs, shards across 8 NeuronCores
(sequence-parallel residual stream + head-sharded QK-merge + vocab-sharded
unembed), runs one SPMD Bass/Tile kernel, reassembles the full output.

v2: fp8(e4m3) tensor cores for the dominant matmuls (scores via DoubleRow,
QK-merge, MLP up-proj, unembed), SBUF-resident unembed weights with per-row-
tile softmax denominators (no DRAM spill of the exp matrix), halved collective
payloads. Activation scales are folded host-side (see _prep_inputs) and undone
in the activation-function `scale` arguments.
"""
import sys

sys.path.insert(0, "/opt/trn_rl_repo")

import numpy as np
import ml_dtypes

import concourse.bass as bass
import concourse.mybir as mybir
import concourse.tile as tile
from concourse import bacc
from concourse.bass_utils import run_bass_kernel_spmd
from concourse.masks import make_identity

F32 = mybir.dt.float32
BF16 = mybir.dt.bfloat16
F8 = mybir.dt.float8e4
AF = mybir.ActivationFunctionType
ALU = mybir.AluOpType
DR = mybir.MatmulPerfMode.DoubleRow

L, D, H, DV, DM, VOC, NL = 2048, 768, 12, 64, 3072, 32000, 4
NC = 8
R = L // NC            # 256 rows per core
VC = VOC // NC         # 4000 vocab cols per core
ET = D // 128          # 6 feature tiles
JT = DM // 128         # 24 mlp tiles
MT = L // 128          # 16 m (key) tiles
LT = R // 128          # 2 local row tiles
NB = VC // 500         # 8 unembed col blocks
SCALE = float(1.0 / np.sqrt(np.float32(D)))

# fp8 pre-scales (host-side; undone in activation `scale` args)
CL = 4.0               # LN1 / final-LN gamma,beta (lT for scores+unembed)
CW = 8.0               # LN2 gamma,beta (znT for MLP)
CM = 8.0               # each of mq, mk  -> M carries CM*CM = 64
CC = 32.0              # combo v-columns (xnTf x4 * 32 = x128 = pvl scale)
CCW = 1024.0           # combo w-columns
CU = 64.0              # unembed / MLP-up weight scale

_CACHE = {}


def _pair(t, w, c, lo, hi):
    """[128, 6*w] et-major tile -> [128, 2, hi-lo] fp8 pair view for
    et = (2c, 2c+1), used as a DoubleRow matmul operand."""
    return t[:, 2 * c * w:(2 * c + 2) * w].rearrange("p (e n) -> p e n", e=2)[:, :, lo:hi]


def _build(analyze=False, reps=1):
    nc = bacc.Bacc("TRN2", target_bir_lowering=False, debug=False, num_devices=NC)

    # ---------------- I/O ----------------
    e0 = nc.dram_tensor("e0", [R, D], F32, kind="ExternalInput")
    mq = nc.dram_tensor("mq", [NL, 3, D, D], F8, kind="ExternalInput")
    mk = nc.dram_tensor("mk", [NL, 3, D, 384], F8, kind="ExternalInput")
    lnp = nc.dram_tensor("lnp", [NL, D, 6], F32, kind="ExternalInput")
    lnf = nc.dram_tensor("lnf", [D, 2], F32, kind="ExternalInput")
    combo = nc.dram_tensor("combo", [NL, D, 87], F8, kind="ExternalInput")
    woe = nc.dram_tensor("woe", [NL, 76, D], BF16, kind="ExternalInput")
    w1 = nc.dram_tensor("w1", [NL, D, DM], F8, kind="ExternalInput")
    bm1c = nc.dram_tensor("bm1c", [NL, 128, JT], F32, kind="ExternalInput")
    w2 = nc.dram_tensor("w2", [NL, DM, D], BF16, kind="ExternalInput")
    bm2r = nc.dram_tensor("bm2r", [NL, 1, D], BF16, kind="ExternalInput")
    wue = nc.dram_tensor("wue", [D, VC], F8, kind="ExternalInput")
    bur = nc.dram_tensor("bur", [1, VC], BF16, kind="ExternalInput")
    out = nc.dram_tensor("out", [L, VC], BF16, kind="ExternalOutput")

    # ---------------- internal DRAM ----------------
    xnt_mine = [nc.dram_tensor(f"xnt_mine{i}", [D, R], F8) for i in range(NL + 1)]
    xnt_all = [
        nc.dram_tensor(f"xnt_all{i}", [NC * D, R], F8, addr_space="Shared")
        for i in range(NL + 1)
    ]
    mcontrib = [nc.dram_tensor(f"mcon{i}", [3 * D, 384], F8) for i in range(NL)]
    m_all = [
        nc.dram_tensor(f"mall{i}", [NC * 3 * D, 384], F8, addr_space="Shared")
        for i in range(NL)
    ]
    denm = [nc.dram_tensor(f"denm{g}", [512], F32) for g in range(4)]
    den_allm = [
        nc.dram_tensor(f"den_allm{g}", [512], F32, addr_space="Shared")
        for g in range(4)
    ]

    RG = [list(range(NC))]

    with tile.TileContext(nc) as tc:
      with (
                tc.tile_pool(name="const", bufs=1) as cpool,
                tc.tile_pool(name="pers", bufs=1) as pers,
                tc.tile_pool(name="work", bufs=2) as work,
                tc.tile_pool(name="wt", bufs=1) as wtp,
                tc.tile_pool(name="mqp", bufs=1) as mqp,
                tc.tile_pool(name="mkp", bufs=2) as mkp,
                tc.tile_pool(name="mhp", bufs=3) as mhp,
                tc.tile_pool(name="etp", bufs=4) as etp,
                tc.tile_pool(name="w1p", bufs=6) as w1p,
                tc.tile_pool(name="w2p", bufs=24) as w2p,
                tc.tile_pool(name="gtp", bufs=24) as gtp,
                tc.tile_pool(name="eup", bufs=40) as eup,
                tc.tile_pool(name="scp", bufs=3) as scp,
                tc.tile_pool(name="ps", bufs=2, space="PSUM") as ps,
                tc.tile_pool(name="ps3", bufs=2, space="PSUM") as ps3,
      ):
        def _one_rep():
                # constants
                ident = cpool.tile([128, 128], BF16)
                make_identity(nc, ident[:])
                identf = cpool.tile([128, 128], F32)
                make_identity(nc, identf[:])
                ones_row = cpool.tile([1, 128], BF16)   # K=1 matmul lhsT (all-ones)
                nc.vector.memset(ones_row[:], 1.0)

                # residual stream, f32: Y[:, lt*D + d], row l = lt*128 + p
                Y = pers.tile([128, LT * D], F32)
                for lt in range(LT):
                    nc.sync.dma_start(Y[:, lt * D:(lt + 1) * D], e0[lt * 128:(lt + 1) * 128, :])

                xnTf = pers.tile([128, ET * L], F8)   # gathered feature-major LN out

                def layernorm(pcol_g, pcol_b, raw_cols, want_rowmajor):
                    """LN of Y chunk -> (lT fp8 [128, ET*R] feature-major, zn bf16 or None).

                    pcol_g/pcol_b: et -> sbuf [128, 1] f32 scaled gamma/beta columns.
                    raw_cols: (g_raw, b_raw) accessors for the unscaled row-major copy.
                    """
                    lT = pers.tile([128, ET * R], F8, tag="lT", name="lT")
                    lTb = pers.tile([128, ET * R], BF16, tag="lTb", name="lTb") if want_rowmajor else None
                    zn = pers.tile([128, LT * D], BF16, tag="zn", name="zn") if want_rowmajor else None
                    for lt in range(LT):
                        ys = Y[:, lt * D:(lt + 1) * D]
                        mean = work.tile([128, 1], F32, tag="m1")
                        nc.vector.reduce_sum(mean[:], ys, axis=mybir.AxisListType.X)
                        nmean = work.tile([128, 1], F32, tag="m2")
                        nc.scalar.mul(nmean[:], mean[:], -1.0 / D)
                        cent = work.tile([128, D], BF16, tag="cent")
                        nc.vector.tensor_scalar_add(cent[:], ys, nmean[:])
                        sq = work.tile([128, D], BF16, tag="sq")
                        ssum = work.tile([128, 1], F32, tag="m3")
                        nc.scalar.activation(sq[:], cent[:], AF.Square, accum_out=ssum[:])
                        var = work.tile([128, 1], F32, tag="m4")
                        nc.scalar.mul(var[:], ssum[:], 1.0 / D)
                        std = work.tile([128, 1], F32, tag="m5")
                        nc.scalar.sqrt(std[:], var[:])
                        rstd = work.tile([128, 1], F32, tag="m6")
                        nc.vector.reciprocal(rstd[:], std[:])
                        norm = work.tile([128, D], BF16, tag="norm")
                        nc.vector.tensor_scalar_mul(norm[:], cent[:], rstd[:])
                        for et in range(ET):
                            pt = ps.tile([128, 128], BF16, tag="tr")
                            nc.tensor.transpose(pt[:], norm[:, et * 128:(et + 1) * 128], ident[:])
                            dst = lT[:, et * R + lt * 128: et * R + (lt + 1) * 128]
                            nc.vector.tensor_scalar(
                                dst, pt[:], pcol_g(et), pcol_b(et),
                                op0=ALU.mult, op1=ALU.add,
                            )
                            if want_rowmajor:
                                g_raw, b_raw = raw_cols
                                dstb = lTb[:, et * R + lt * 128: et * R + (lt + 1) * 128]
                                nc.vector.tensor_scalar(
                                    dstb, pt[:], g_raw(et), b_raw(et),
                                    op0=ALU.mult, op1=ALU.add,
                                )
                    if want_rowmajor:
                        for lt in range(LT):
                            for et in range(ET):
                                pt = ps.tile([128, 128], BF16, tag="tr")
                                nc.tensor.transpose(
                                    pt[:], lTb[:, et * R + lt * 128: et * R + (lt + 1) * 128],
                                    ident[:],
                                )
                                nc.vector.tensor_copy(zn[:, lt * D + et * 128: lt * D + (et + 1) * 128], pt[:])
                    return lT, zn

                def gather_lt(lT, mine_dram, all_dram):
                    """DMA local feature-major chunk to DRAM, AllGather, load full."""
                    for et in range(ET):
                        nc.sync.dma_start(
                            mine_dram[et * 128:(et + 1) * 128, :],
                            lT[:, et * R:(et + 1) * R],
                        )
                    if analyze:
                        nc.sync.dma_start(all_dram[0:D, :], mine_dram[:])
                    else:
                        nc.gpsimd.collective_compute(
                            "AllGather", ALU.bypass, replica_groups=RG,
                            ins=[mine_dram[:]], outs=[all_dram[:]],
                        )
                    # all_dram rows: c*D + et*128 + p, cols l_local -> xnTf[p, et*L + c*R + l]
                    v = all_dram[:, :].rearrange("(c e p) l -> e p c l", c=NC, e=ET, p=128)
                    for et in range(ET):
                        dst = xnTf[:, et * L:(et + 1) * L].rearrange("p (c l) -> p c l", c=NC)
                        nc.sync.dma_start(dst, v[et])

                def compute_tT(i, h, hh, tT, lT):
                    mh_sb = []
                    for s in range(2):
                        g = 2 * h + s
                        kcore, p3 = g // 3, g % 3
                        mhs = mhp.tile([128, ET * 384], F8, tag="mh", name="mhs")
                        base = kcore * 3 * D + p3 * D
                        nc.sync.dma_start(
                            mhs[:].rearrange("p (e c) -> p e c", c=384),
                            m_all[i][base:base + D, :].rearrange("(e p) c -> p e c", p=128),
                        )
                        mh_sb.append(mhs)
                    for n6 in range(ET):
                        mhs = mh_sb[n6 // 3]
                        c0 = (n6 % 3) * 128
                        tp = ps3.tile([128, R], F32, tag="smm", name="tp")
                        for et in range(ET):
                            nc.tensor.matmul(
                                tp[:], mhs[:, et * 384 + c0: et * 384 + c0 + 128],
                                lT[:, et * R:(et + 1) * R],
                                start=(et == 0), stop=(et == ET - 1),
                            )
                        nc.vector.tensor_copy(
                            tT[:, n6 * 2 * R + hh * R: n6 * 2 * R + (hh + 1) * R], tp[:]
                        )

                def compute_m(i):
                    for p3 in range(3):
                        mkt = mkp.tile([128, ET * 384], F8, tag="mk", name="mkt")
                        nc.sync.dma_start(
                            mkt[:].rearrange("p (e c) -> p e c", c=384),
                            mk[i, p3].rearrange("(e p) c -> p e c", p=128),
                        )
                        qt = mqp.tile([128, ET * D], F8, tag="mq", name="qt")
                        nc.sync.dma_start(
                            qt[:].rearrange("p (e c) -> p e c", c=D),
                            mq[i, p3].rearrange("(e p) c -> p e c", p=128),
                        )
                        for mt6 in range(ET):
                            mp = ps.tile([128, 384], F32, tag="mm", name="mp")
                            for kc in range(3):
                                nc.tensor.matmul(
                                    mp[:], _pair(qt, D, kc, mt6 * 128, (mt6 + 1) * 128),
                                    _pair(mkt, 384, kc, 0, 384),
                                    start=(kc == 0), stop=(kc == 2), perf_mode=DR,
                                )
                            mo = work.tile([128, 384], F8, tag="mo", name="mo")
                            nc.vector.tensor_copy(mo[:], mp[:])
                            nc.sync.dma_start(
                                mcontrib[i][p3 * D + mt6 * 128: p3 * D + (mt6 + 1) * 128, :],
                                mo[:],
                            )
                    if analyze:
                        nc.sync.dma_start(m_all[i][0:3 * D, :], mcontrib[i][:])
                    else:
                        nc.gpsimd.collective_compute(
                            "AllGather", ALU.bypass, replica_groups=RG,
                            ins=[mcontrib[i][:]], outs=[m_all[i][:]],
                        )

                # layer 0's M first (its AG gates layer 0 attention), then LN1[0]
                # chunk + its AG, then prefetch M for layers 1..3 (their AGs drain
                # while layer 0 computes).
                compute_m(0)

                # ================= layers =================
                for i in range(NL):
                    lnpt = wtp.tile([128, ET * 6], F32, tag="lnp")
                    for et in range(ET):
                        nc.sync.dma_start(
                            lnpt[:, et * 6:(et + 1) * 6], lnp[i, et * 128:(et + 1) * 128, :]
                        )
                    g1c = lambda et: lnpt[:, et * 6 + 0: et * 6 + 1]
                    b1c = lambda et: lnpt[:, et * 6 + 1: et * 6 + 2]
                    g2c = lambda et: lnpt[:, et * 6 + 2: et * 6 + 3]
                    b2c = lambda et: lnpt[:, et * 6 + 3: et * 6 + 4]
                    g2r = lambda et: lnpt[:, et * 6 + 4: et * 6 + 5]
                    b2r = lambda et: lnpt[:, et * 6 + 5: et * 6 + 6]

                    # ---- LN1 -> local feature-major + allgather ----
                    lT, _ = layernorm(g1c, b1c, None, want_rowmajor=False)
                    gather_lt(lT, xnt_mine[i], xnt_all[i])
                    if i + 1 < NL:
                        compute_m(i + 1)

                    # ---- combo: w (12) | v0 (11) | V11 (64) over all m ----
                    cmb = wtp.tile([128, ET * 87], F8, tag="cmb")
                    for et in range(ET):
                        nc.sync.dma_start(
                            cmb[:, et * 87:(et + 1) * 87], combo[i, et * 128:(et + 1) * 128, :]
                        )
                    w_sb = pers.tile([128, MT * 12], F32, tag="wsb")
                    pvl = pers.tile([128, MT * 80], F8, tag="pvl")  # stride 80: dual-fp8 LDW needs step%16==0
                    for mt in range(MT):
                        cp = ps.tile([128, 87], F32, tag="pv")
                        for et in range(ET):
                            nc.tensor.matmul(
                                cp[:], xnTf[:, et * L + mt * 128: et * L + (mt + 1) * 128],
                                cmb[:, et * 87:(et + 1) * 87],
                                start=(et == 0), stop=(et == ET - 1),
                            )
                        nc.scalar.mul(w_sb[:, mt * 12:(mt + 1) * 12], cp[:, 0:12], 1.0 / (CCW * CL))
                        nc.vector.tensor_copy(pvl[:, mt * 80: mt * 80 + 75], cp[:, 12:87])
                        nc.vector.memset(pvl[:, mt * 80 + 75: mt * 80 + 76], 128.0)

                    # ---- attention heads ----
                    ylm = pers.tile([128, LT * 76], BF16, tag="ylm")  # l-major y + ones col
                    for lt in range(LT):
                        nc.vector.memset(ylm[:, lt * 76 + 75: lt * 76 + 76], 1.0)
                    woet = wtp.tile([76, D], BF16, tag="woe")
                    nc.sync.dma_start(woet[:], woe[i])

                    for hpair in range(H // 2):
                        # pair heads (2hp, 2hp+1): shared lhsT in the S matmul ->
                        # one N=512 rhs, fp8 DoubleRow halves the instruction count.
                        heads = (2 * hpair, 2 * hpair + 1)
                        tT = work.tile([128, ET * 2 * R], F8, tag="tT", bufs=1, name="tT")
                        for hh, h in enumerate(heads):
                            compute_tT(i, h, hh, tT, lT)
                        pvs = [ps.tile([128, R], F32, tag="pv", name="pv") for _ in range(2)]
                        prev_eTs = None
                        for mp in range(MT // 2):
                            eT2s = [etp.tile([128, 2 * R], F8, tag="eTm", name="eT2")
                                    for _ in range(2)]
                            for sub in range(2):
                                mt = 2 * mp + sub
                                sp = ps3.tile([128, 2 * R], F32, tag="smm", name="sp")
                                for kc in range(3):
                                    nc.tensor.matmul(
                                        sp[:], _pair(xnTf, L, kc, mt * 128, (mt + 1) * 128),
                                        _pair(tT, 2 * R, kc, 0, 2 * R),
                                        start=(kc == 0), stop=(kc == 2), perf_mode=DR,
                                    )
                                for hh, h in enumerate(heads):
                                    nc.scalar.activation(
                                        eT2s[hh][:, sub * R:(sub + 1) * R],
                                        sp[:, hh * R:(hh + 1) * R], AF.Exp,
                                        bias=w_sb[:, mt * 12 + h: mt * 12 + h + 1],
                                        scale=SCALE / (CL * CL * CM * CM),
                                    )
                            # PV (fp8 DR over 256-key pairs) delayed one mt-pair so
                            # the exps are done when PE gets here
                            if prev_eTs is not None:
                                for hh in range(2):
                                    nc.tensor.matmul(
                                        pvs[hh][0:76, :],
                                        _pair(pvl, 80, mp - 1, 0, 76),
                                        prev_eTs[hh][:].rearrange("p (e n) -> p e n", e=2),
                                        start=(mp - 1 == 0), stop=False, perf_mode=DR,
                                    )
                            prev_eTs = eT2s
                        for hh in range(2):
                            nc.tensor.matmul(
                                pvs[hh][0:76, :],
                                _pair(pvl, 80, MT // 2 - 1, 0, 76),
                                prev_eTs[hh][:].rearrange("p (e n) -> p e n", e=2),
                                start=False, stop=True, perf_mode=DR,
                            )
                        for hh, h in enumerate(heads):
                            pv_sb = work.tile([76, R], F32, tag="pvsb", name="pv_sb")
                            nc.vector.tensor_copy(pv_sb[:], pvs[hh][0:76, :])
                            for lt in range(LT):
                                pvT = ps.tile([128, 76], F32, tag="tr", name="pvT")
                                nc.tensor.transpose(
                                    pvT[:], pv_sb[:, lt * 128:(lt + 1) * 128], identf[0:76, 0:76]
                                )
                                recip = work.tile([128, 1], F32, tag="recip", name="recip")
                                nc.vector.reciprocal(recip[:], pvT[:, 75:76])
                                if h < H - 1:
                                    nc.vector.tensor_scalar_mul(
                                        ylm[:, lt * 76 + h: lt * 76 + h + 1],
                                        pvT[:, h:h + 1], recip[:],
                                    )
                                else:
                                    nc.vector.tensor_scalar_mul(
                                        ylm[:, lt * 76 + 11: lt * 76 + 75],
                                        pvT[:, 11:75], recip[:],
                                    )

                    # ---- out-proj + residual: Y = 2Y + yT.T @ [Wo;bo] ----
                    yT = pers.tile([76, LT * 128], BF16, tag="yT")
                    for lt in range(LT):
                        ytp = ps.tile([128, 128], BF16, tag="tr")
                        nc.tensor.transpose(
                            ytp[0:76, :], ylm[:, lt * 76:(lt + 1) * 76], ident[:]
                        )
                        nc.vector.tensor_copy(yT[:, lt * 128:(lt + 1) * 128], ytp[0:76, :])
                    for lt in range(LT):
                        for nb2 in range(2):
                            ap = ps.tile([128, 384], F32, tag="mm")
                            nc.tensor.matmul(
                                ap[:], yT[:, lt * 128:(lt + 1) * 128],
                                woet[:, nb2 * 384:(nb2 + 1) * 384],
                                start=True, stop=True,
                            )
                            ysl = Y[:, lt * D + nb2 * 384: lt * D + (nb2 + 1) * 384]
                            nc.vector.scalar_tensor_tensor(
                                ysl, ysl, 2.0, ap[:], op0=ALU.mult, op1=ALU.add
                            )

                    # ---- MLP ----
                    znT, zn = layernorm(g2c, b2c, (g2r, b2r), want_rowmajor=True)
                    w1t = [w1p.tile([128, DM], F8, tag="w1", name="w1t") for _ in range(ET)]
                    for et in range(ET):
                        nc.sync.dma_start(w1t[et][:], w1[i, et * 128:(et + 1) * 128, :])
                    bm1t = wtp.tile([128, JT], F32, tag="bm1")
                    nc.sync.dma_start(bm1t[:], bm1c[i])
                    gts = []
                    for jt in range(JT):
                        hp = ps.tile([128, R], F32, tag="mm")
                        for et in range(ET):
                            nc.tensor.matmul(
                                hp[:], w1t[et][:, jt * 128:(jt + 1) * 128],
                                znT[:, et * R:(et + 1) * R],
                                start=(et == 0), stop=(et == ET - 1),
                            )
                        gt = gtp.tile([128, R], BF16, tag="gT")
                        nc.scalar.activation(
                            gt[:], hp[:], AF.Gelu_apprx_tanh,
                            bias=bm1t[:, jt:jt + 1], scale=1.0 / (CW * CU),
                        )
                        gts.append(gt)
                    w2t = [w2p.tile([128, D], BF16, tag="w2", name="w2t") for _ in range(JT)]
                    for jt in range(JT):
                        nc.sync.dma_start(w2t[jt][:], w2[i, jt * 128:(jt + 1) * 128, :])
                    bm2t = wtp.tile([1, D], BF16, tag="bm2")
                    nc.sync.dma_start(bm2t[:], bm2r[i])
                    for lt in range(LT):
                        nc.vector.tensor_add(
                            Y[:, lt * D:(lt + 1) * D], Y[:, lt * D:(lt + 1) * D],
                            zn[:, lt * D:(lt + 1) * D],
                        )
                        for nb2 in range(2):
                            mp2 = ps.tile([128, 384], F32, tag="mm")
                            for jt in range(JT):
                                nc.tensor.matmul(
                                    mp2[:], gts[jt][:, lt * 128:(lt + 1) * 128],
                                    w2t[jt][:, nb2 * 384:(nb2 + 1) * 384],
                                    start=(jt == 0), stop=False,
                                )
                            nc.tensor.matmul(
                                mp2[:], ones_row[:, 0:128],
                                bm2t[:, nb2 * 384:(nb2 + 1) * 384],
                                start=False, stop=True,
                            )
                            ysl = Y[:, lt * D + nb2 * 384: lt * D + (nb2 + 1) * 384]
                            nc.vector.tensor_add(ysl, ysl, mp2[:])

                # ============ final LN + unembed + fused softmax ============
                lnft = wtp.tile([128, ET * 2], F32, tag="lnf")
                for et in range(ET):
                    nc.sync.dma_start(lnft[:, et * 2:(et + 1) * 2], lnf[et * 128:(et + 1) * 128, :])
                gfc = lambda et: lnft[:, et * 2 + 0: et * 2 + 1]
                bfc = lambda et: lnft[:, et * 2 + 1: et * 2 + 2]
                # unembed weights resident in SBUF (fp8), single pass over Wu;
                # issued before the final LN so the loads overlap it
                wu_sb = pers.tile([128, ET * VC], F8, tag="wusb")
                for et in range(ET):
                    nc.sync.dma_start(wu_sb[:, et * VC:(et + 1) * VC], wue[et * 128:(et + 1) * 128, :])
                but = wtp.tile([1, VC], BF16, tag="bu")
                nc.sync.dma_start(but[:], bur[:])

                lT, _ = layernorm(gfc, bfc, None, want_rowmajor=False)
                gather_lt(lT, xnt_mine[NL], xnt_all[NL])

                for grp in range(4):
                    grp_eus = {}
                    for j in range(4):
                        mt = grp * 4 + j
                        dpart = work.tile([128, NB], F32, tag="dpart", name="dpart")
                        eus = []
                        for nb in range(NB):
                            up = ps3.tile([128, 500], F32, tag="smm", name="up")
                            for kc in range(3):
                                nc.tensor.matmul(
                                    up[:], _pair(xnTf, L, kc, mt * 128, (mt + 1) * 128),
                                    _pair(wu_sb, VC, kc, nb * 500, (nb + 1) * 500),
                                    start=(kc == 0), stop=False, perf_mode=DR,
                                )
                            nc.tensor.matmul(
                                up[:], ones_row[:, 0:128], but[:, nb * 500:(nb + 1) * 500],
                                start=False, stop=True,
                            )
                            eu = eup.tile([128, 500], BF16, tag="eu", name="eu")
                            nc.scalar.activation(
                                eu[:], up[:], AF.Exp, scale=1.0 / (CL * CU),
                                accum_out=dpart[:, nb:nb + 1],
                            )
                            eus.append(eu)
                        grp_eus[j] = eus
                        dloc = work.tile([128, 1], F32, tag="dloc", name="dloc")
                        nc.vector.reduce_sum(dloc[:], dpart[:], axis=mybir.AxisListType.X)
                        nc.sync.dma_start(denm[grp][j * 128:(j + 1) * 128], dloc[:])
                    if analyze:
                        nc.sync.dma_start(den_allm[grp][:], denm[grp][:])
                    else:
                        nc.gpsimd.collective_compute(
                            "AllReduce", ALU.add, replica_groups=RG,
                            ins=[denm[grp][:]], outs=[den_allm[grp][:]],
                        )
                    dall = work.tile([128, 4], F32, tag="dall", name="dall")
                    nc.sync.dma_start(dall[:], den_allm[grp][:].rearrange("(m p) -> p m", p=128))
                    drec = work.tile([128, 4], F32, tag="drec", name="drec")
                    nc.vector.reciprocal(drec[:], dall[:])
                    for j in range(4):
                        mt = grp * 4 + j
                        for nb in range(NB):
                            sout = scp.tile([128, 500], BF16, tag="so", name="sout")
                            nc.vector.tensor_scalar_mul(
                                sout[:], grp_eus[j][nb][:], drec[:, j:j + 1]
                            )
                            nc.sync.dma_start(
                                out[mt * 128:(mt + 1) * 128, nb * 500:(nb + 1) * 500], sout[:]
                            )

        for _rep in range(reps):
            _one_rep()

    nc.compile()
    return nc


def _prep_inputs(inputs):
    bf = ml_dtypes.bfloat16
    f8 = mybir.dt.np(F8)
    x = np.asarray(inputs["x"])
    E0 = (np.asarray(inputs["word_embed"])[x] + np.asarray(inputs["pos_embed"])).astype(np.float32)
    Wq, bq = np.asarray(inputs["Wq"]), np.asarray(inputs["bq"])
    Wk = np.asarray(inputs["Wk"])
    Wv, bv = np.asarray(inputs["Wv"]), np.asarray(inputs["bv"])
    Wo, bo = np.asarray(inputs["Wo"]), np.asarray(inputs["bo"])
    W1, bm1 = np.asarray(inputs["W1"]), np.asarray(inputs["bm1"])
    W2, bm2 = np.asarray(inputs["W2"]), np.asarray(inputs["bm2"])
    Wu, bu = np.asarray(inputs["Wu"]), np.asarray(inputs["bu"])

    g1 = np.asarray(inputs["g1"]); be1 = np.asarray(inputs["be1"])
    g2 = np.asarray(inputs["g2"]); be2 = np.asarray(inputs["be2"])
    lnp = np.stack(
        [CL * g1, CL * be1, CW * g2, CW * be2, g2, be2], axis=-1
    ).astype(np.float32)                                   # [NL, D, 6]
    lnf = np.stack(
        [CL * np.asarray(inputs["gf"]), CL * np.asarray(inputs["bef"])], -1
    ).astype(np.float32)

    combo = np.zeros((NL, D, 87), np.float32)
    for i in range(NL):
        for h in range(H):
            combo[i, :, h] = (Wk[i, h] @ bq[i, h]) * SCALE * CCW  # u_scaled
        combo[i, :, 12:23] = Wv[i, :11, :, 0].transpose(1, 0) * CC
        combo[i, :, 23:87] = Wv[i, 11] * CC
    woe = np.zeros((NL, 76, D), np.float32)
    for i in range(NL):
        bv_flat = np.concatenate([bv[i, :11, 0], bv[i, 11]])
        woe[i, :75] = Wo[i, :75]
        woe[i, 75] = bo[i] + bv_flat @ Wo[i, :75]
    bm1c = bm1.reshape(NL, JT, 128).transpose(0, 2, 1).astype(np.float32)

    in_maps = []
    for k in range(NC):
        mqk = np.zeros((NL, 3, D, D), np.float32)
        mkk = np.zeros((NL, 3, D, 384), np.float32)
        for p3 in range(3):
            g = 3 * k + p3
            h, s = g // 2, g % 2
            for i in range(NL):
                mqk[i, p3] = CM * Wq[i, h].T
                mkk[i, p3] = CM * Wk[i, h][s * 384:(s + 1) * 384, :].T
        in_maps.append({
            "e0": E0[k * R:(k + 1) * R],
            "mq": mqk.astype(f8),
            "mk": mkk.astype(f8),
            "lnp": lnp,
            "lnf": lnf,
            "combo": combo.astype(f8),
            "woe": woe.astype(bf),
            "w1": (CU * W1).astype(f8),
            "bm1c": bm1c,
            "w2": W2.astype(bf),
            "bm2r": bm2.reshape(NL, 1, D).astype(bf),
            "wue": np.ascontiguousarray(CU * Wu[:, k * VC:(k + 1) * VC]).astype(f8),
            "bur": np.ascontiguousarray(CL * CU * bu[None, k * VC:(k + 1) * VC]).astype(bf),
        })
    return in_maps


def _run(inputs, **kw):
    if "nc" not in _CACHE:
        _CACHE["nc"] = _build()
    nc = _CACHE["nc"]
    in_maps = _prep_inputs(inputs)
    res = run_bass_kernel_spmd(nc, in_maps, list(range(NC)), **kw)
    outp = np.concatenate([res.results[k]["out"] for k in range(NC)], axis=1)
    return outp.astype(np.float32), res


def kernel(**inputs):
    outp, _ = _run(inputs)
    return outp


# revision 16
# speedup vs baseline: 2.3335x; 1.0067x over previous
"""Trainium2 Bass kernel for the 4-layer dense transformer (nn_DTransformer).

Self-contained: takes full unsharded inputs, shards across 8 NeuronCores
(sequence-parallel residual stream + head-sharded QK-merge + vocab-sharded
unembed), runs one SPMD Bass/Tile kernel, reassembles the full output.

v2: fp8(e4m3) tensor cores for the dominant matmuls (scores via DoubleRow,
QK-merge, MLP up-proj, unembed), SBUF-resident unembed weights with per-row-
tile softmax denominators (no DRAM spill of the exp matrix), halved collective
payloads. Activation scales are folded host-side (see _prep_inputs) and undone
in the activation-function `scale` arguments.
"""
import sys

sys.path.insert(0, "/opt/trn_rl_repo")

import numpy as np
import ml_dtypes

import concourse.bass as bass
import concourse.mybir as mybir
import concourse.tile as tile
from concourse import bacc
from concourse.bass_utils import run_bass_kernel_spmd
from concourse.masks import make_identity

F32 = mybir.dt.float32
BF16 = mybir.dt.bfloat16
F8 = mybir.dt.float8e4
AF = mybir.ActivationFunctionType
ALU = mybir.AluOpType
DR = mybir.MatmulPerfMode.DoubleRow

L, D, H, DV, DM, VOC, NL = 2048, 768, 12, 64, 3072, 32000, 4
NC = 8
R = L // NC            # 256 rows per core
VC = VOC // NC         # 4000 vocab cols per core
ET = D // 128          # 6 feature tiles
JT = DM // 128         # 24 mlp tiles
MT = L // 128          # 16 m (key) tiles
LT = R // 128          # 2 local row tiles
NB = VC // 500         # 8 unembed col blocks
SCALE = float(1.0 / np.sqrt(np.float32(D)))

# fp8 pre-scales (host-side; undone in activation `scale` args)
CL = 4.0               # LN1 / final-LN gamma,beta (lT for scores+unembed)
CW = 8.0               # LN2 gamma,beta (znT for MLP)
CM = 8.0               # each of mq, mk  -> M carries CM*CM = 64
CC = 32.0              # combo v-columns (xnTf x4 * 32 = x128 = pvl scale)
CCW = 1024.0           # combo w-columns
CU = 64.0              # unembed / MLP-up weight scale

_CACHE = {}


def _pair(t, w, c, lo, hi):
    """[128, 6*w] et-major tile -> [128, 2, hi-lo] fp8 pair view for
    et = (2c, 2c+1), used as a DoubleRow matmul operand."""
    return t[:, 2 * c * w:(2 * c + 2) * w].rearrange("p (e n) -> p e n", e=2)[:, :, lo:hi]


def _build(analyze=False, reps=1):
    nc = bacc.Bacc("TRN2", target_bir_lowering=False, debug=False, num_devices=NC)

    # ---------------- I/O ----------------
    e0 = nc.dram_tensor("e0", [R, D], F32, kind="ExternalInput")
    mq = nc.dram_tensor("mq", [NL, 3, D, D], F8, kind="ExternalInput")
    mk = nc.dram_tensor("mk", [NL, 3, D, 384], F8, kind="ExternalInput")
    lnp = nc.dram_tensor("lnp", [NL, D, 6], F32, kind="ExternalInput")
    lnf = nc.dram_tensor("lnf", [D, 2], F32, kind="ExternalInput")
    combo = nc.dram_tensor("combo", [NL, D, 87], F8, kind="ExternalInput")
    woe = nc.dram_tensor("woe", [NL, 76, D], BF16, kind="ExternalInput")
    w1 = nc.dram_tensor("w1", [NL, D, DM], F8, kind="ExternalInput")
    bm1c = nc.dram_tensor("bm1c", [NL, 128, JT], F32, kind="ExternalInput")
    w2 = nc.dram_tensor("w2", [NL, DM, D], BF16, kind="ExternalInput")
    bm2r = nc.dram_tensor("bm2r", [NL, 1, D], BF16, kind="ExternalInput")
    wue = nc.dram_tensor("wue", [D, VC], F8, kind="ExternalInput")
    bur = nc.dram_tensor("bur", [1, VC], BF16, kind="ExternalInput")
    out = nc.dram_tensor("out", [L, VC], BF16, kind="ExternalOutput")

    # ---------------- internal DRAM ----------------
    xnt_mine = [nc.dram_tensor(f"xnt_mine{i}", [D, R], F8) for i in range(NL + 1)]
    xnt_all = [
        nc.dram_tensor(f"xnt_all{i}", [NC * D, R], F8, addr_space="Shared")
        for i in range(NL + 1)
    ]
    mcontrib = [nc.dram_tensor(f"mcon{i}", [384, ET * 384], F8) for i in range(NL)]
    m_all = [
        nc.dram_tensor(f"mall{i}", [NC * 384, ET * 384], F8, addr_space="Shared")
        for i in range(NL)
    ]
    denm = [nc.dram_tensor(f"denm{g}", [512], F32) for g in range(4)]
    den_allm = [
        nc.dram_tensor(f"den_allm{g}", [512], F32, addr_space="Shared")
        for g in range(4)
    ]

    RG = [list(range(NC))]

    with tile.TileContext(nc) as tc:
      with (
                tc.tile_pool(name="const", bufs=1) as cpool,
                tc.tile_pool(name="pers", bufs=1) as pers,
                tc.tile_pool(name="work", bufs=2) as work,
                tc.tile_pool(name="wt", bufs=1) as wtp,
                tc.tile_pool(name="mqp", bufs=1) as mqp,
                tc.tile_pool(name="mkp", bufs=2) as mkp,
                tc.tile_pool(name="mhp", bufs=3) as mhp,
                tc.tile_pool(name="etp", bufs=4) as etp,
                tc.tile_pool(name="w1p", bufs=6) as w1p,
                tc.tile_pool(name="w2p", bufs=24) as w2p,
                tc.tile_pool(name="gtp", bufs=24) as gtp,
                tc.tile_pool(name="eup", bufs=40) as eup,
                tc.tile_pool(name="scp", bufs=3) as scp,
                tc.tile_pool(name="ps", bufs=2, space="PSUM") as ps,
                tc.tile_pool(name="ps3", bufs=2, space="PSUM") as ps3,
      ):
        def _one_rep():
                # constants
                ident = cpool.tile([128, 128], BF16)
                make_identity(nc, ident[:])
                identf = cpool.tile([128, 128], F32)
                make_identity(nc, identf[:])
                ones_row = cpool.tile([1, 128], BF16)   # K=1 matmul lhsT (all-ones)
                nc.vector.memset(ones_row[:], 1.0)

                # residual stream, f32: Y[:, lt*D + d], row l = lt*128 + p
                Y = pers.tile([128, LT * D], F32)
                for lt in range(LT):
                    nc.sync.dma_start(Y[:, lt * D:(lt + 1) * D], e0[lt * 128:(lt + 1) * 128, :])

                xnTf = pers.tile([128, ET * L], F8)   # gathered feature-major LN out

                def layernorm(pcol_g, pcol_b, raw_cols, want_rowmajor):
                    """LN of Y chunk -> (lT fp8 [128, ET*R] feature-major, zn bf16 or None).

                    pcol_g/pcol_b: et -> sbuf [128, 1] f32 scaled gamma/beta columns.
                    raw_cols: (g_raw, b_raw) accessors for the unscaled row-major copy.
                    """
                    lT = pers.tile([128, ET * R], F8, tag="lT", name="lT")
                    lTb = pers.tile([128, ET * R], BF16, tag="lTb", name="lTb") if want_rowmajor else None
                    zn = pers.tile([128, LT * D], BF16, tag="zn", name="zn") if want_rowmajor else None
                    for lt in range(LT):
                        ys = Y[:, lt * D:(lt + 1) * D]
                        mean = work.tile([128, 1], F32, tag="m1")
                        nc.vector.reduce_sum(mean[:], ys, axis=mybir.AxisListType.X)
                        nmean = work.tile([128, 1], F32, tag="m2")
                        nc.scalar.mul(nmean[:], mean[:], -1.0 / D)
                        cent = work.tile([128, D], BF16, tag="cent")
                        nc.vector.tensor_scalar_add(cent[:], ys, nmean[:])
                        sq = work.tile([128, D], BF16, tag="sq")
                        ssum = work.tile([128, 1], F32, tag="m3")
                        nc.scalar.activation(sq[:], cent[:], AF.Square, accum_out=ssum[:])
                        var = work.tile([128, 1], F32, tag="m4")
                        nc.scalar.mul(var[:], ssum[:], 1.0 / D)
                        std = work.tile([128, 1], F32, tag="m5")
                        nc.scalar.sqrt(std[:], var[:])
                        rstd = work.tile([128, 1], F32, tag="m6")
                        nc.vector.reciprocal(rstd[:], std[:])
                        norm = work.tile([128, D], BF16, tag="norm")
                        nc.vector.tensor_scalar_mul(norm[:], cent[:], rstd[:])
                        for et in range(ET):
                            pt = ps.tile([128, 128], BF16, tag="tr")
                            nc.tensor.transpose(pt[:], norm[:, et * 128:(et + 1) * 128], ident[:])
                            dst = lT[:, et * R + lt * 128: et * R + (lt + 1) * 128]
                            nc.vector.tensor_scalar(
                                dst, pt[:], pcol_g(et), pcol_b(et),
                                op0=ALU.mult, op1=ALU.add,
                            )
                            if want_rowmajor:
                                g_raw, b_raw = raw_cols
                                dstb = lTb[:, et * R + lt * 128: et * R + (lt + 1) * 128]
                                nc.vector.tensor_scalar(
                                    dstb, pt[:], g_raw(et), b_raw(et),
                                    op0=ALU.mult, op1=ALU.add,
                                )
                    if want_rowmajor:
                        for lt in range(LT):
                            for et in range(ET):
                                pt = ps.tile([128, 128], BF16, tag="tr")
                                nc.tensor.transpose(
                                    pt[:], lTb[:, et * R + lt * 128: et * R + (lt + 1) * 128],
                                    ident[:],
                                )
                                nc.vector.tensor_copy(zn[:, lt * D + et * 128: lt * D + (et + 1) * 128], pt[:])
                    return lT, zn

                def gather_lt(lT, mine_dram, all_dram):
                    """DMA local feature-major chunk to DRAM, AllGather, load full."""
                    for et in range(ET):
                        nc.sync.dma_start(
                            mine_dram[et * 128:(et + 1) * 128, :],
                            lT[:, et * R:(et + 1) * R],
                        )
                    if analyze:
                        nc.sync.dma_start(all_dram[0:D, :], mine_dram[:])
                    else:
                        nc.gpsimd.collective_compute(
                            "AllGather", ALU.bypass, replica_groups=RG,
                            ins=[mine_dram[:]], outs=[all_dram[:]],
                        )
                    # all_dram rows: c*D + et*128 + p, cols l_local -> xnTf[p, et*L + c*R + l]
                    v = all_dram[:, :].rearrange("(c e p) l -> e p c l", c=NC, e=ET, p=128)
                    for et in range(ET):
                        dst = xnTf[:, et * L:(et + 1) * L].rearrange("p (c l) -> p c l", c=NC)
                        nc.sync.dma_start(dst, v[et])

                def compute_tT(i, h, hh, tT, lT):
                    mh_sb = []
                    for s in range(2):
                        g = 2 * h + s
                        kcore, p3 = g // 3, g % 3
                        mhs = mhp.tile([128, ET * 384], F8, tag="mh", name="mhs")
                        base = (kcore * 3 + p3) * 128
                        nc.sync.dma_start(mhs[:], m_all[i][base:base + 128, :])
                        mh_sb.append(mhs)
                    for n6 in range(ET):
                        mhs = mh_sb[n6 // 3]
                        c0 = (n6 % 3) * 128
                        tp = ps3.tile([128, R], F32, tag="smm", name="tp")
                        for et in range(ET):
                            nc.tensor.matmul(
                                tp[:], mhs[:, et * 384 + c0: et * 384 + c0 + 128],
                                lT[:, et * R:(et + 1) * R],
                                start=(et == 0), stop=(et == ET - 1),
                            )
                        nc.vector.tensor_copy(
                            tT[:, n6 * 2 * R + hh * R: n6 * 2 * R + (hh + 1) * R], tp[:]
                        )

                def compute_m(i):
                    for p3 in range(3):
                        mkt = mkp.tile([128, ET * 384], F8, tag="mk", name="mkt")
                        nc.sync.dma_start(
                            mkt[:].rearrange("p (e c) -> p e c", c=384),
                            mk[i, p3].rearrange("(e p) c -> p e c", p=128),
                        )
                        qt = mqp.tile([128, ET * D], F8, tag="mq", name="qt")
                        nc.sync.dma_start(
                            qt[:].rearrange("p (e c) -> p e c", c=D),
                            mq[i, p3].rearrange("(e p) c -> p e c", p=128),
                        )
                        for mt6 in range(ET):
                            mp = ps.tile([128, 384], F32, tag="mm", name="mp")
                            for kc in range(3):
                                nc.tensor.matmul(
                                    mp[:], _pair(qt, D, kc, mt6 * 128, (mt6 + 1) * 128),
                                    _pair(mkt, 384, kc, 0, 384),
                                    start=(kc == 0), stop=(kc == 2), perf_mode=DR,
                                )
                            mo = work.tile([128, 384], F8, tag="mo", name="mo")
                            nc.vector.tensor_copy(mo[:], mp[:])
                            nc.sync.dma_start(
                                mcontrib[i][p3 * 128:(p3 + 1) * 128, mt6 * 384:(mt6 + 1) * 384],
                                mo[:],
                            )
                    if analyze:
                        nc.sync.dma_start(m_all[i][0:384, :], mcontrib[i][:])
                    else:
                        nc.gpsimd.collective_compute(
                            "AllGather", ALU.bypass, replica_groups=RG,
                            ins=[mcontrib[i][:]], outs=[m_all[i][:]],
                        )

                # layer 0's M first (its AG gates layer 0 attention), then LN1[0]
                # chunk + its AG, then prefetch M for layers 1..3 (their AGs drain
                # while layer 0 computes).
                compute_m(0)

                # ================= layers =================
                for i in range(NL):
                    lnpt = wtp.tile([128, ET * 6], F32, tag="lnp")
                    for et in range(ET):
                        nc.sync.dma_start(
                            lnpt[:, et * 6:(et + 1) * 6], lnp[i, et * 128:(et + 1) * 128, :]
                        )
                    g1c = lambda et: lnpt[:, et * 6 + 0: et * 6 + 1]
                    b1c = lambda et: lnpt[:, et * 6 + 1: et * 6 + 2]
                    g2c = lambda et: lnpt[:, et * 6 + 2: et * 6 + 3]
                    b2c = lambda et: lnpt[:, et * 6 + 3: et * 6 + 4]
                    g2r = lambda et: lnpt[:, et * 6 + 4: et * 6 + 5]
                    b2r = lambda et: lnpt[:, et * 6 + 5: et * 6 + 6]

                    # ---- LN1 -> local feature-major + allgather ----
                    lT, _ = layernorm(g1c, b1c, None, want_rowmajor=False)
                    gather_lt(lT, xnt_mine[i], xnt_all[i])
                    if i + 1 < NL:
                        compute_m(i + 1)

                    # ---- combo: w (12) | v0 (11) | V11 (64) over all m ----
                    cmb = wtp.tile([128, ET * 87], F8, tag="cmb")
                    for et in range(ET):
                        nc.sync.dma_start(
                            cmb[:, et * 87:(et + 1) * 87], combo[i, et * 128:(et + 1) * 128, :]
                        )
                    w_sb = pers.tile([128, MT * 12], F32, tag="wsb")
                    pvl = pers.tile([128, MT * 80], F8, tag="pvl")  # stride 80: dual-fp8 LDW needs step%16==0
                    for mt in range(MT):
                        cp = ps.tile([128, 87], F32, tag="pv")
                        for et in range(ET):
                            nc.tensor.matmul(
                                cp[:], xnTf[:, et * L + mt * 128: et * L + (mt + 1) * 128],
                                cmb[:, et * 87:(et + 1) * 87],
                                start=(et == 0), stop=(et == ET - 1),
                            )
                        nc.scalar.mul(w_sb[:, mt * 12:(mt + 1) * 12], cp[:, 0:12], 1.0 / (CCW * CL))
                        nc.vector.tensor_copy(pvl[:, mt * 80: mt * 80 + 75], cp[:, 12:87])
                        nc.vector.memset(pvl[:, mt * 80 + 75: mt * 80 + 76], 128.0)

                    # ---- attention heads ----
                    ylm = pers.tile([128, LT * 76], BF16, tag="ylm")  # l-major y + ones col
                    for lt in range(LT):
                        nc.vector.memset(ylm[:, lt * 76 + 75: lt * 76 + 76], 1.0)
                    woet = wtp.tile([76, D], BF16, tag="woe")
                    nc.sync.dma_start(woet[:], woe[i])

                    for hpair in range(H // 2):
                        # pair heads (2hp, 2hp+1): shared lhsT in the S matmul ->
                        # one N=512 rhs, fp8 DoubleRow halves the instruction count.
                        heads = (2 * hpair, 2 * hpair + 1)
                        tT = work.tile([128, ET * 2 * R], F8, tag="tT", bufs=1, name="tT")
                        for hh, h in enumerate(heads):
                            compute_tT(i, h, hh, tT, lT)
                        pvs = [ps.tile([128, R], F32, tag="pv", name="pv") for _ in range(2)]
                        prev_eTs = None
                        for mp in range(MT // 2):
                            eT2s = [etp.tile([128, 2 * R], F8, tag="eTm", name="eT2")
                                    for _ in range(2)]
                            for sub in range(2):
                                mt = 2 * mp + sub
                                sp = ps3.tile([128, 2 * R], F32, tag="smm", name="sp")
                                for kc in range(3):
                                    nc.tensor.matmul(
                                        sp[:], _pair(xnTf, L, kc, mt * 128, (mt + 1) * 128),
                                        _pair(tT, 2 * R, kc, 0, 2 * R),
                                        start=(kc == 0), stop=(kc == 2), perf_mode=DR,
                                    )
                                for hh, h in enumerate(heads):
                                    nc.scalar.activation(
                                        eT2s[hh][:, sub * R:(sub + 1) * R],
                                        sp[:, hh * R:(hh + 1) * R], AF.Exp,
                                        bias=w_sb[:, mt * 12 + h: mt * 12 + h + 1],
                                        scale=SCALE / (CL * CL * CM * CM),
                                    )
                            # PV (fp8 DR over 256-key pairs) delayed one mt-pair so
                            # the exps are done when PE gets here
                            if prev_eTs is not None:
                                for hh in range(2):
                                    nc.tensor.matmul(
                                        pvs[hh][0:76, :],
                                        _pair(pvl, 80, mp - 1, 0, 76),
                                        prev_eTs[hh][:].rearrange("p (e n) -> p e n", e=2),
                                        start=(mp - 1 == 0), stop=False, perf_mode=DR,
                                    )
                            prev_eTs = eT2s
                        for hh in range(2):
                            nc.tensor.matmul(
                                pvs[hh][0:76, :],
                                _pair(pvl, 80, MT // 2 - 1, 0, 76),
                                prev_eTs[hh][:].rearrange("p (e n) -> p e n", e=2),
                                start=False, stop=True, perf_mode=DR,
                            )
                        for hh, h in enumerate(heads):
                            pv_sb = work.tile([76, R], F32, tag="pvsb", name="pv_sb")
                            nc.vector.tensor_copy(pv_sb[:], pvs[hh][0:76, :])
                            for lt in range(LT):
                                pvT = ps.tile([128, 76], F32, tag="tr", name="pvT")
                                nc.tensor.transpose(
                                    pvT[:], pv_sb[:, lt * 128:(lt + 1) * 128], identf[0:76, 0:76]
                                )
                                recip = work.tile([128, 1], F32, tag="recip", name="recip")
                                nc.vector.reciprocal(recip[:], pvT[:, 75:76])
                                if h < H - 1:
                                    nc.vector.tensor_scalar_mul(
                                        ylm[:, lt * 76 + h: lt * 76 + h + 1],
                                        pvT[:, h:h + 1], recip[:],
                                    )
                                else:
                                    nc.vector.tensor_scalar_mul(
                                        ylm[:, lt * 76 + 11: lt * 76 + 75],
                                        pvT[:, 11:75], recip[:],
                                    )

                    # ---- out-proj + residual: Y = 2Y + yT.T @ [Wo;bo] ----
                    yT = pers.tile([76, LT * 128], BF16, tag="yT")
                    for lt in range(LT):
                        ytp = ps.tile([128, 128], BF16, tag="tr")
                        nc.tensor.transpose(
                            ytp[0:76, :], ylm[:, lt * 76:(lt + 1) * 76], ident[:]
                        )
                        nc.vector.tensor_copy(yT[:, lt * 128:(lt + 1) * 128], ytp[0:76, :])
                    for lt in range(LT):
                        for nb2 in range(2):
                            ap = ps.tile([128, 384], F32, tag="mm")
                            nc.tensor.matmul(
                                ap[:], yT[:, lt * 128:(lt + 1) * 128],
                                woet[:, nb2 * 384:(nb2 + 1) * 384],
                                start=True, stop=True,
                            )
                            ysl = Y[:, lt * D + nb2 * 384: lt * D + (nb2 + 1) * 384]
                            nc.vector.scalar_tensor_tensor(
                                ysl, ysl, 2.0, ap[:], op0=ALU.mult, op1=ALU.add
                            )

                    # ---- MLP ----
                    znT, zn = layernorm(g2c, b2c, (g2r, b2r), want_rowmajor=True)
                    w1t = [w1p.tile([128, DM], F8, tag="w1", name="w1t") for _ in range(ET)]
                    for et in range(ET):
                        nc.sync.dma_start(w1t[et][:], w1[i, et * 128:(et + 1) * 128, :])
                    bm1t = wtp.tile([128, JT], F32, tag="bm1")
                    nc.sync.dma_start(bm1t[:], bm1c[i])
                    gts = []
                    for jt in range(JT):
                        hp = ps.tile([128, R], F32, tag="mm")
                        for et in range(ET):
                            nc.tensor.matmul(
                                hp[:], w1t[et][:, jt * 128:(jt + 1) * 128],
                                znT[:, et * R:(et + 1) * R],
                                start=(et == 0), stop=(et == ET - 1),
                            )
                        gt = gtp.tile([128, R], BF16, tag="gT")
                        nc.scalar.activation(
                            gt[:], hp[:], AF.Gelu_apprx_tanh,
                            bias=bm1t[:, jt:jt + 1], scale=1.0 / (CW * CU),
                        )
                        gts.append(gt)
                    w2t = [w2p.tile([128, D], BF16, tag="w2", name="w2t") for _ in range(JT)]
                    for jt in range(JT):
                        nc.sync.dma_start(w2t[jt][:], w2[i, jt * 128:(jt + 1) * 128, :])
                    bm2t = wtp.tile([1, D], BF16, tag="bm2")
                    nc.sync.dma_start(bm2t[:], bm2r[i])
                    for lt in range(LT):
                        nc.vector.tensor_add(
                            Y[:, lt * D:(lt + 1) * D], Y[:, lt * D:(lt + 1) * D],
                            zn[:, lt * D:(lt + 1) * D],
                        )
                        for nb2 in range(2):
                            mp2 = ps.tile([128, 384], F32, tag="mm")
                            for jt in range(JT):
                                nc.tensor.matmul(
                                    mp2[:], gts[jt][:, lt * 128:(lt + 1) * 128],
                                    w2t[jt][:, nb2 * 384:(nb2 + 1) * 384],
                                    start=(jt == 0), stop=False,
                                )
                            nc.tensor.matmul(
                                mp2[:], ones_row[:, 0:128],
                                bm2t[:, nb2 * 384:(nb2 + 1) * 384],
                                start=False, stop=True,
                            )
                            ysl = Y[:, lt * D + nb2 * 384: lt * D + (nb2 + 1) * 384]
                            nc.vector.tensor_add(ysl, ysl, mp2[:])

                # ============ final LN + unembed + fused softmax ============
                lnft = wtp.tile([128, ET * 2], F32, tag="lnf")
                for et in range(ET):
                    nc.sync.dma_start(lnft[:, et * 2:(et + 1) * 2], lnf[et * 128:(et + 1) * 128, :])
                gfc = lambda et: lnft[:, et * 2 + 0: et * 2 + 1]
                bfc = lambda et: lnft[:, et * 2 + 1: et * 2 + 2]
                # unembed weights resident in SBUF (fp8), single pass over Wu;
                # issued before the final LN so the loads overlap it
                wu_sb = pers.tile([128, ET * VC], F8, tag="wusb")
                for et in range(ET):
                    nc.sync.dma_start(wu_sb[:, et * VC:(et + 1) * VC], wue[et * 128:(et + 1) * 128, :])
                but = wtp.tile([1, VC], BF16, tag="bu")
                nc.sync.dma_start(but[:], bur[:])

                lT, _ = layernorm(gfc, bfc, None, want_rowmajor=False)
                gather_lt(lT, xnt_mine[NL], xnt_all[NL])

                for grp in range(4):
                    grp_eus = {}
                    for j in range(4):
                        mt = grp * 4 + j
                        dpart = work.tile([128, NB], F32, tag="dpart", name="dpart")
                        eus = []
                        for nb in range(NB):
                            up = ps3.tile([128, 500], F32, tag="smm", name="up")
                            for kc in range(3):
                                nc.tensor.matmul(
                                    up[:], _pair(xnTf, L, kc, mt * 128, (mt + 1) * 128),
                                    _pair(wu_sb, VC, kc, nb * 500, (nb + 1) * 500),
                                    start=(kc == 0), stop=False, perf_mode=DR,
                                )
                            nc.tensor.matmul(
                                up[:], ones_row[:, 0:128], but[:, nb * 500:(nb + 1) * 500],
                                start=False, stop=True,
                            )
                            eu = eup.tile([128, 500], BF16, tag="eu", name="eu")
                            nc.scalar.activation(
                                eu[:], up[:], AF.Exp, scale=1.0 / (CL * CU),
                                accum_out=dpart[:, nb:nb + 1],
                            )
                            eus.append(eu)
                        grp_eus[j] = eus
                        dloc = work.tile([128, 1], F32, tag="dloc", name="dloc")
                        nc.vector.reduce_sum(dloc[:], dpart[:], axis=mybir.AxisListType.X)
                        nc.sync.dma_start(denm[grp][j * 128:(j + 1) * 128], dloc[:])
                    if analyze:
                        nc.sync.dma_start(den_allm[grp][:], denm[grp][:])
                    else:
                        nc.gpsimd.collective_compute(
                            "AllReduce", ALU.add, replica_groups=RG,
                            ins=[denm[grp][:]], outs=[den_allm[grp][:]],
                        )
                    dall = work.tile([128, 4], F32, tag="dall", name="dall")
                    nc.sync.dma_start(dall[:], den_allm[grp][:].rearrange("(m p) -> p m", p=128))
                    drec = work.tile([128, 4], F32, tag="drec", name="drec")
                    nc.vector.reciprocal(drec[:], dall[:])
                    for j in range(4):
                        mt = grp * 4 + j
                        for nb in range(NB):
                            sout = scp.tile([128, 500], BF16, tag="so", name="sout")
                            nc.vector.tensor_scalar_mul(
                                sout[:], grp_eus[j][nb][:], drec[:, j:j + 1]
                            )
                            nc.sync.dma_start(
                                out[mt * 128:(mt + 1) * 128, nb * 500:(nb + 1) * 500], sout[:]
                            )

        for _rep in range(reps):
            _one_rep()

    nc.compile()
    return nc


def _prep_inputs(inputs):
    bf = ml_dtypes.bfloat16
    f8 = mybir.dt.np(F8)
    x = np.asarray(inputs["x"])
    E0 = (np.asarray(inputs["word_embed"])[x] + np.asarray(inputs["pos_embed"])).astype(np.float32)
    Wq, bq = np.asarray(inputs["Wq"]), np.asarray(inputs["bq"])
    Wk = np.asarray(inputs["Wk"])
    Wv, bv = np.asarray(inputs["Wv"]), np.asarray(inputs["bv"])
    Wo, bo = np.asarray(inputs["Wo"]), np.asarray(inputs["bo"])
    W1, bm1 = np.asarray(inputs["W1"]), np.asarray(inputs["bm1"])
    W2, bm2 = np.asarray(inputs["W2"]), np.asarray(inputs["bm2"])
    Wu, bu = np.asarray(inputs["Wu"]), np.asarray(inputs["bu"])

    g1 = np.asarray(inputs["g1"]); be1 = np.asarray(inputs["be1"])
    g2 = np.asarray(inputs["g2"]); be2 = np.asarray(inputs["be2"])
    lnp = np.stack(
        [CL * g1, CL * be1, CW * g2, CW * be2, g2, be2], axis=-1
    ).astype(np.float32)                                   # [NL, D, 6]
    lnf = np.stack(
        [CL * np.asarray(inputs["gf"]), CL * np.asarray(inputs["bef"])], -1
    ).astype(np.float32)

    combo = np.zeros((NL, D, 87), np.float32)
    for i in range(NL):
        for h in range(H):
            combo[i, :, h] = (Wk[i, h] @ bq[i, h]) * SCALE * CCW  # u_scaled
        combo[i, :, 12:23] = Wv[i, :11, :, 0].transpose(1, 0) * CC
        combo[i, :, 23:87] = Wv[i, 11] * CC
    woe = np.zeros((NL, 76, D), np.float32)
    for i in range(NL):
        bv_flat = np.concatenate([bv[i, :11, 0], bv[i, 11]])
        woe[i, :75] = Wo[i, :75]
        woe[i, 75] = bo[i] + bv_flat @ Wo[i, :75]
    bm1c = bm1.reshape(NL, JT, 128).transpose(0, 2, 1).astype(np.float32)

    in_maps = []
    for k in range(NC):
        mqk = np.zeros((NL, 3, D, D), np.float32)
        mkk = np.zeros((NL, 3, D, 384), np.float32)
        for p3 in range(3):
            g = 3 * k + p3
            h, s = g // 2, g % 2
            for i in range(NL):
                mqk[i, p3] = CM * Wq[i, h].T
                mkk[i, p3] = CM * Wk[i, h][s * 384:(s + 1) * 384, :].T
        in_maps.append({
            "e0": E0[k * R:(k + 1) * R],
            "mq": mqk.astype(f8),
            "mk": mkk.astype(f8),
            "lnp": lnp,
            "lnf": lnf,
            "combo": combo.astype(f8),
            "woe": woe.astype(bf),
            "w1": (CU * W1).astype(f8),
            "bm1c": bm1c,
            "w2": W2.astype(bf),
            "bm2r": bm2.reshape(NL, 1, D).astype(bf),
            "wue": np.ascontiguousarray(CU * Wu[:, k * VC:(k + 1) * VC]).astype(f8),
            "bur": np.ascontiguousarray(CL * CU * bu[None, k * VC:(k + 1) * VC]).astype(bf),
        })
    return in_maps


def _run(inputs, **kw):
    if "nc" not in _CACHE:
        _CACHE["nc"] = _build()
    nc = _CACHE["nc"]
    in_maps = _prep_inputs(inputs)
    res = run_bass_kernel_spmd(nc, in_maps, list(range(NC)), **kw)
    outp = np.concatenate([res.results[k]["out"] for k in range(NC)], axis=1)
    return outp.astype(np.float32), res


def kernel(**inputs):
    outp, _ = _run(inputs)
    return outp
